# revision 55
# baseline (speedup 1.0000x reference)
"""MAE ViT encoder (nn_MaskedAutoencoderViT) Trainium2 Bass kernel.

Strategy: data-parallel over batch (16 images -> 8 cores x 2 images).
Feature-major activation layout on chip: activations stored transposed as
[128 partitions (d chunk), 6 chunks, 152 tokens] so every matmul is
weight-stationary (lhsT = 128x128 weight tile, rhs = activation columns)
with zero on-device transposes.  Attention is computed in transposed form
(S^T = (K^T)-stationary @ Q^T), softmax uses the structure
exp(att)/ (sum + 1e-9) (the reference's global-max subtraction cancels in
the normalization up to ~1e-10 relative, far below fp32 noise).
Matmul operands in fp16 (full PE rate, 11-bit mantissa), accumulation and
residual stream in fp32.

Scheduling structure (all serialization chains measured in the timeline
cost model):
- weight streaming: one DMA per k-chunk / fc2-quarter; DMA issue and the
  shared HWDGE device serialize per-instruction, so chunk-merged DMAs
  keep both off the critical path (transfer cost is bytes-based).
- each residual drain runs twice in parallel: Pool produces the fp32
  stream (H, double-buffered), DVE produces the fp16 matmul operand x16
  directly into the LN staging tile; ACT squares follow per chunk, so
  LN stats and the next GEMM both unblock ~0.6us after the psum stop.
- LN mean/rstd travel to all partitions via a 1x128 ones-matmul into
  PSUM (~130ns on the idle-ish PE) instead of gpsimd partition_broadcast
  (~1.4us on Pool).
- QKV and fc1 run on raw x16 with a K=1 colsum*mu correction row; V runs
  on raw x16 too, with the correction applied token-major and rstd folded
  into the per-token mask scale (needs one 1-col PE transpose).
- exp/gelu activation-table loads are hoisted off the critical path by
  dummy [1,1] activations issued while ACT is idle.
Host side does only data marshalling: noise argsort, patch gather,
pos-embed gathers, weight transposition + fp16 cast.
"""
import numpy as np
from contextlib import ExitStack

import concourse.bass as bass
import concourse.bacc as bacc
import concourse.mybir as mybir
import concourse.tile as tile
import bass_rust as _bass_rust
from concourse.bass_utils import run_bass_kernel_spmd
from concourse.hw_specs import get_activation_tables


class _Bacc(bacc.Bacc):
    """Bacc whose ACT-table-load pass prefers multi-function sets.

    The stock pass picks the first table set containing each activation
    function, which sends Ln to `natural_log` and Exp to `exp_and_others`
    and thrashes the table RAM inside every layernorm.  Reordering the
    set dict so `natural_log_exp_and_others` comes first makes Ln and Exp
    share one resident set (2 loads per layer total: exp-set <-> gelu-set).
    """

    def insert_act_table_loads(self):
        has_activation = any(
            isinstance(i, mybir.InstActivation)
            for b in self.main_func.blocks
            for i in b.instructions
        )
        if not has_activation:
            return
        # Keep the canonical set order (set ids are positional and the
        # runtime resolves them canonically) but hide Exp/Ln from every
        # other set so the chooser lands on the combined one.
        tabs = get_activation_tables(self.m.arch)
        items = []
        for k, v in tabs.items():
            if k != "natural_log_exp_and_others":
                v = {f for f in v if f.name not in ("Exp", "Ln")}
            items.append((k, v))
        _bass_rust.insert_act_table_loads(self, items)

F16 = mybir.dt.float16
F32 = mybir.dt.float32
AF = mybir.ActivationFunctionType
OP = mybir.AluOpType

# --- model config (hardcoded from the problem spec) ---
B, C_IN, H_IN, W_IN = 16, 1, 12, 2500
P_, Q_ = 1, 100
D, NH, DEPTH = 768, 12, 12
GH, GW = 12, 25
L = GH * GW                      # 300
LEN_KEEP = 75
HD = D // NH                     # 64
SCALE = HD ** -0.5               # 0.125
EPS_LN = 1e-5
MLP = 4 * D                      # 3072

NCORES = 8
BL = B // NCORES                 # 2 images per core
KT = 1 + LEN_KEEP                # 76 tokens per image
T = BL * KT                      # 152 token columns per core
NCH = D // 128                   # 6 feature chunks
MCH = MLP // 128                 # 24 mlp chunks
PIX = P_ * Q_                    # 100 pixels per patch


def bfree(ap, n, at=1):
    """Insert a 0-step (broadcast) free dim of size n at position `at`."""
    new_ap = list(ap.ap[:at]) + [[0, n]] + list(ap.ap[at:])
    return bass.AP(tensor=ap.tensor, offset=ap.offset, ap=new_ap)


def build(depth=DEPTH):
    nc = _Bacc("TRN2", target_bir_lowering=False, debug=False,
               num_devices=NCORES)

    # DRAM I/O
    patchesT = nc.dram_tensor("patchesT", [PIX, T], F16, kind="ExternalInput").ap()
    posT = nc.dram_tensor("posT", [NCH, 128, T], F32, kind="ExternalInput").ap()
    mvec = nc.dram_tensor("mvec", [BL, KT], F16, kind="ExternalInput").ap()
    mvecf = nc.dram_tensor("mvecf", [1, BL, 2, 6 * KT], F16, kind="ExternalInput").ap()
    wpatchT = nc.dram_tensor("wpatchT", [PIX, D], F16, kind="ExternalInput").ap()
    wqkvT = nc.dram_tensor("wqkvT", [depth, D, 3 * D], F16,
                           kind="ExternalInput").ap()
    wprojT = nc.dram_tensor("wprojT", [depth, D, D], F16,
                            kind="ExternalInput").ap()
    wfc1T = nc.dram_tensor("wfc1T", [depth, D, MLP], F16, kind="ExternalInput").ap()
    wfc2T = nc.dram_tensor("wfc2T", [depth, MLP, D], F16, kind="ExternalInput").ap()
    # [0:2D]: -colsum(Wq|Wk); [2D:3D]: -colsum(Wv); [3D:]: -colsum(Wfc1),
    # packed two partition rows of 2688 so the tile costs half the columns
    wvecs = nc.dram_tensor("wvecs", [depth, 2, 2688], F16,
                           kind="ExternalInput").ap()
    out_d = nc.dram_tensor("out", [128, NCH, T], F16, kind="ExternalOutput").ap()

    with tile.TileContext(nc) as tc, ExitStack() as ctx:
        pool = lambda name, bufs, **kw: ctx.enter_context(
            tc.tile_pool(name=name, bufs=bufs, **kw))

        const = pool("const", 1)
        hp = pool("hp", 2)
        lnp = pool("lnp", 2)
        qkp = pool("qkp", 1)
        vp = pool("vp", 2)
        ep = pool("ep", 2)
        otp = pool("otp", 1)
        gp = pool("gp", 1)
        tinyp = pool("tinyp", 5)
        medp = pool("medp", 3)
        bcp = pool("bcp", 2)
        wsump = pool("wsump", 2)
        wqkvp = pool("wqkvp", 7)
        wprojp = pool("wprojp", 7)
        wfc1p = pool("wfc1p", 14)
        wfc2p = pool("wfc2p", 6)

        psB = pool("psB", 3, space="PSUM")
        psC = pool("psC", 4, space="PSUM")
        pab = pool("pab", 1, space="PSUM")

        # constants
        ones16 = const.tile([128, 1], F16)
        nc.vector.memset(ones16[:], 1.0)
        onesr = const.tile([1, 64], F16)
        nc.vector.memset(onesr[:], 1.0)
        onesr128 = const.tile([1, 128], F16)
        nc.vector.memset(onesr128[:], 1.0)
        eps_t = const.tile([1, 1], F32)
        nc.vector.memset(eps_t[:], EPS_LN)

        # static inputs; the patch-embed-only tiles borrow weight-pool
        # slots (they die right after the patch embed, so the slots return
        # to the steady-state weight stream)
        patches_sb = wfc1p.tile([PIX, T], F16, tag="wfc1", name="patches_sb")
        nc.sync.dma_start(out=patches_sb[:], in_=patchesT[:])
        wpatch_sb = wfc1p.tile([PIX, D], F16, tag="wfc1", name="wpatch_sb")
        nc.sync.dma_start(out=wpatch_sb[:], in_=wpatchT[:])
        pos_sb = wfc2p.tile([128, NCH, T], F32, tag="wfc2", name="pos_sb")
        nc.sync.dma_start(out=pos_sb[:], in_=posT.rearrange("c p t -> p c t"))
        m_sb = const.tile([KT, BL], F16)
        nc.sync.dma_start(out=m_sb[:], in_=mvec.rearrange("b t -> t b"))
        m32_sb = const.tile([KT, BL], F32)
        nc.vector.tensor_copy(m32_sb[:], m_sb[:])
        mf_sb = const.tile([1, BL, 2, 6 * KT], F16)
        nc.sync.dma_start(out=mf_sb[:], in_=mvecf[:])

        def drain_x16(ps_ap, g, lnin_new, src_ap):
            """fp16 matmul-operand cast for one 3-chunk group + its square
            (ACT for group 0, fp16-2x DVE for group 1, so they overlap).
            Emitted right after that group's psum stop so the LN chain
            starts before the other bank finishes accumulating."""
            sl = slice(3 * g, 3 * g + 3)
            nc.vector.tensor_add(lnin_new[:, 0, sl, :], src_ap, ps_ap)
            if g == 0:
                nc.scalar.activation(lnin_new[:, 1, sl, :],
                                     lnin_new[:, 0, sl, :], AF.Square)
            else:
                nc.vector.tensor_mul(lnin_new[:, 1, sl, :],
                                     lnin_new[:, 0, sl, :],
                                     lnin_new[:, 0, sl, :])

        def drain_H(ps_aps, h_new, src_aps):
            """fp32 residual-stream update; not needed until the next
            boundary, so it runs after both x16 casts.  Pool has no PSUM
            port on TRN2, so these adds also live on DVE."""
            for g in range(2):
                nc.vector.tensor_add(h_new[:, 3 * g:3 * g + 3, :], src_aps[g],
                                     ps_aps[g])

        def ln_stats(lnin):
            """lnin [128,2,NCH,T] fp16 prefilled with [x | x^2].
            Returns (mu16 [1,T] f16, ab psum [128,2,T] f32 with
            [:,0]=rstd, [:,1]=mu broadcast, anb16 [1,2,T] f16)."""
            st = psC.tile([1, 2, T], F32, tag="psC", name="st")
            for c in range(NCH):
                nc.tensor.matmul(st[:], ones16[:, 0:1], lnin[:, :, c, :],
                                 start=(c == 0), stop=(c == NCH - 1))
            mean = tinyp.tile([1, T], F32, tag="tiny")
            nc.vector.tensor_scalar_mul(mean[:], st[0:1, 0, :], 1.0 / D)
            # mu16 feeds the K=1 corrections that close the running psum
            # groups -- emit straight after the mean, before the rstd chain
            mu16 = tinyp.tile([1, T], F16, tag="tiny16")
            nc.vector.tensor_copy(mu16[:], mean[:])
            mu32 = tinyp.tile([33, T], F16, tag="mu2")
            nc.gpsimd.partition_broadcast(mu32[:], mu16[:])
            msq = tinyp.tile([1, T], F32, tag="tiny")
            nc.vector.tensor_mul(msq[:], mean[:], mean[:])
            v = tinyp.tile([1, T], F32, tag="tiny")
            nc.vector.scalar_tensor_tensor(v[:], st[0:1, 1, :], 1.0 / D, msq[:],
                                           op0=OP.mult, op1=OP.subtract)
            # rstd = exp(-0.5*ln(v+eps)) on ACT: Ln and Exp live in the
            # same table set (see _Bacc), and this replaces the 8-op DVE
            # Newton chain (each [1,T] op pays ~150ns of sem latency)
            anb16 = medp.tile([1, 2, T], F16, tag="anb")
            lnv = tinyp.tile([1, T], F32, tag="tiny")
            nc.scalar.activation(lnv[:], v[:], AF.Ln, bias=eps_t[0:1, 0:1])
            nc.scalar.activation(anb16[0:1, 0, :], lnv[:], AF.Exp, scale=-0.5)
            nc.vector.tensor_copy(anb16[0:1, 1, :], mean[:])
            return (mu16, mu32), anb16

        def ln_bcast(anb16):
            """Broadcast [1,2,T] across partitions via PE ones-matmul,
            then ACT-copy PSUM->SBUF (the drains' other operand is PSUM,
            and only one PSUM operand per DVE op is legal)."""
            ab = pab.tile([128, 2, T], F32, tag="pab")
            nc.tensor.matmul(ab[:], onesr128[:], anb16[:],
                             start=True, stop=True)
            ab_sb = bcp.tile([128, 2, T], F16, tag="bc")
            nc.scalar.activation(ab_sb[:], ab[:], AF.Copy)
            return ab_sb

        # residual stream, feature-major fp32 (double-buffered so the
        # Pool-side update overlaps the DVE-side fp16 cast)
        H = hp.tile([128, NCH, T], F32, tag="H")
        lnin1 = lnp.tile([128, 2, NCH, T], F16, tag="lnin")

        # ---- patch embed + pos add ----
        pe_ps = []
        for grp in range(2):
            ps3 = psB.tile([128, 3, T], F32, tag="psB", name="pe3")
            for i in range(3):
                c = 3 * grp + i
                nc.tensor.matmul(ps3[:, i, :], wpatch_sb[:, 128 * c:128 * (c + 1)],
                                 patches_sb[:], start=(i == 0), stop=(i == 2))
            pe_ps.append(ps3)
        for grp in range(2):
            drain_x16(pe_ps[grp][:, :, :], grp, lnin1,
                      pos_sb[:, 3 * grp:3 * grp + 3, :])
        drain_H([p[:, :, :] for p in pe_ps], H,
                [pos_sb[:, 0:3, :], pos_sb[:, 3:6, :]])

        for l in range(depth):
            # weight loads for this layer (emitted first so DMA starts early)
            wvec_t = wsump.tile([33, 2688], F16, tag="wvec")
            nc.sync.dma_start(out=wvec_t[0:33:32, :], in_=wvecs[l])
            # DMA queue is FIFO; emit in expected buffer-free order so a
            # gate-blocked load never shadows an already-loadable one:
            # spare-slot loads first, then by when the prior layer's tiles
            # die (qkv ~35%, proj ~52%, fc1a ~62%, fc1b ~75%, fc2 late).
            wqkv = [wqkvp.tile([128, 3 * D], F16, tag="wqkv", name="wqkv")
                    for _ in range(NCH)]
            wfc1 = [[wfc1p.tile([128, MLP // 2], F16, tag="wfc1", name="wfc1")
                     for _ in range(NCH)] for _ in range(2)]
            wproj = [wprojp.tile([128, D], F16, tag="wproj", name="wproj")
                     for _ in range(NCH)]
            wfc2_t = [wfc2p.tile([128, 6, D], F16, tag="wfc2", name="wfc2")
                      for _ in range(4)]

            def dma_qkv(k):
                nc.sync.dma_start(out=wqkv[k][:],
                                  in_=wqkvT[l, 128 * k:128 * (k + 1), :])

            def dma_fc1(h, k):
                nc.sync.dma_start(out=wfc1[h][k][:],
                                  in_=wfc1T[l, 128 * k:128 * (k + 1),
                                            1536 * h:1536 * (h + 1)])

            def dma_proj(k):
                nc.sync.dma_start(out=wproj[k][:],
                                  in_=wprojT[l, 128 * k:128 * (k + 1), :])

            def dma_fc2(q):
                nc.sync.dma_start(
                    out=wfc2_t[q][:],
                    in_=wfc2T[l, 768 * q:768 * (q + 1), :].rearrange(
                        "(k p) j -> p k j", p=128))

            for k in range(NCH):
                dma_qkv(k)
            dma_fc1(0, 0)
            dma_fc1(0, 1)
            dma_fc2(0)
            dma_fc2(1)
            for k in range(NCH):
                dma_proj(k)
            for k in range(2, NCH):
                dma_fc1(0, k)
            for k in range(NCH):
                dma_fc1(1, k)
            dma_fc2(2)
            dma_fc2(3)

            # ---- LN1 stats (x and x^2 already staged in lnin1) ----
            (mu16_1, mu32_1), anb16_1 = ln_stats(lnin1)

            # ---- QKV: Q,K feature-major on raw x16 + K=1 correction ----
            qk16 = qkp.tile([128, 2 * NCH, T], F16, tag="qk")
            ab1 = None
            first = True
            for grp in [0, 2, 1, 3]:
                ps3 = psB.tile([128, 3, T], F32, tag="psB", name="qk3")
                for i in range(3):
                    oc = 3 * grp + i
                    for k in range(NCH):
                        nc.tensor.matmul(ps3[:, i, :],
                                         wqkv[k][:, 128 * oc:128 * (oc + 1)],
                                         lnin1[:, 0, k, :],
                                         start=(k == 0), stop=False)
                    nc.tensor.matmul(ps3[:, i, :],
                                     wvec_t[0:1, 128 * oc:128 * (oc + 1)],
                                     mu16_1[:], start=False, stop=True)
                if first:
                    # emitted after the first group's matmuls: the PE sits
                    # behind them in queue order, so the wait on the DVE
                    # stats chain overlaps that GEMM stream
                    ab1 = ln_bcast(anb16_1)
                    first = False
                nc.vector.tensor_mul(qk16[:, 3 * grp:3 * (grp + 1), :],
                                     ps3[:, :, :], bfree(ab1[:, 0, :], 3))

            # per-image rstd as a column (PE transpose) * mask -> V scale
            csc = []
            for b in range(BL):
                vt = psC.tile([KT, 1], F16, tag="psC", name="vt")
                nc.tensor.matmul(vt[:], anb16_1[0:1, 0, KT * b:KT * (b + 1)],
                                 ones16[0:1, 0:1], is_transpose=True)
                cs = tinyp.tile([KT, 1], F32, tag="csc")
                nc.vector.tensor_mul(cs[:], vt[:], m32_sb[:, b:b + 1])
                csc.append(cs)

            # ---- V token-major per image, raw x16 + correction ----
            v16 = []
            for b in range(BL):
                vps0 = psC.tile([KT, 384], F32, tag="psC", name="vps")
                vps1 = psC.tile([KT, 384], F32, tag="psC", name="vps")
                for k in range(NCH):
                    nc.tensor.matmul(vps0[:],
                                     lnin1[:, 0, k, KT * b:KT * (b + 1)],
                                     wqkv[k][:, 2 * D:2 * D + 384],
                                     start=(k == 0), stop=False)
                nc.tensor.matmul(vps0[:], mu16_1[0:1, KT * b:KT * (b + 1)],
                                 wvec_t[0:1, 2 * D:2 * D + 384],
                                 start=False, stop=True)
                for k in range(NCH):
                    nc.tensor.matmul(vps1[:],
                                     lnin1[:, 0, k, KT * b:KT * (b + 1)],
                                     wqkv[k][:, 2 * D + 384:3 * D],
                                     start=(k == 0), stop=False)
                nc.tensor.matmul(vps1[:], mu16_1[0:1, KT * b:KT * (b + 1)],
                                 wvec_t[0:1, 2 * D + 384:3 * D],
                                 start=False, stop=True)
                v = vp.tile([KT, D], F16, tag="v")
                nc.vector.tensor_scalar_mul(v[:, 0:384], vps0[:], csc[b][:])
                nc.vector.tensor_scalar_mul(v[:, 384:768], vps1[:], csc[b][:])
                v16.append(v)

            # ---- attention; images interleaved, heads grouped by parity.
            # PV runs on the raw exp(S) values: the attn mask is folded into
            # the V drain (row scale) and the 1/rowsum normalization into the
            # ot16 assembly multiply, so the softmax scalar chain never
            # blocks the PE stream.
            # fully pipelined per-(image, head-group) units: each unit's
            # exp->rowsum->recip->mask->bcast->assembly chain hides behind
            # the later units' S/PV matmul stream.  No softmax eps: rowsums
            # of exp() are bounded >= e^-O(1) here, and the reference's
            # +1e-9 shifts them by ~1e-11 relative.
            ot16 = otp.tile([128, NCH, T], F16, tag="ot")
            e16s = [ep.tile([KT, 2, 6 * KT], F16, tag="e", name="e16")
                    for _ in range(BL)]
            rrs = [medp.tile([1, 2, 6 * KT], F16, tag="rr", name="rr")
                   for _ in range(BL)]
            rrbs = [bcp.tile([64, 2, 6 * KT], F16, tag="rb", name="rrb")
                    for _ in range(BL)]
            units = [(b, g) for b in range(BL) for g in range(2)]

            def att_s(b, g):
                sps = psC.tile([KT, 6 * KT], F32, tag="psC", name="sps")
                for j in range(6):
                    nc.tensor.matmul(
                        sps[:, KT * j:KT * (j + 1)],
                        qk16[64 * g:64 * (g + 1), 6 + j, KT * b:KT * (b + 1)],
                        qk16[64 * g:64 * (g + 1), j, KT * b:KT * (b + 1)],
                        start=True, stop=True)
                nc.scalar.activation(e16s[b][:, g, :], sps[:],
                                     AF.Exp, scale=SCALE)

            def att_norm(b, g):
                rps = psC.tile([1, 6 * KT], F32, tag="psC", name="rps")
                nc.tensor.matmul(rps[:], m_sb[:, b:b + 1],
                                 e16s[b][:, g, :], start=True, stop=True)
                with nc.allow_low_precision(reason="softmax norm fp16"):
                    nc.vector.reciprocal(rrs[b][0:1, g, :], rps[:])

            def att_out(b, g):
                ops = psC.tile([64, 6 * KT], F32, tag="psC", name="ops")
                for j in range(6):
                    nc.tensor.matmul(
                        ops[:, KT * j:KT * (j + 1)],
                        v16[b][:, 128 * j + 64 * g:128 * j + 64 * g + 64],
                        e16s[b][:, g, KT * j:KT * (j + 1)],
                        start=True, stop=True)
                rbp = psC.tile([64, 6 * KT], F32, tag="psC", name="rbp")
                nc.tensor.matmul(rbp[:], onesr[:], rrs[b][0:1, g, :],
                                 start=True, stop=True)
                nc.scalar.activation(rrbs[b][:, g, :], rbp[:], AF.Copy)
                nc.vector.tensor_mul(
                    ot16[64 * g:64 * (g + 1), :, KT * b:KT * (b + 1)],
                    ops[:].rearrange("p (j t) -> p j t", j=6),
                    rrbs[b][:, g, :].rearrange("p (j t) -> p j t", j=6))

            # 3-stage software pipeline: slot i runs S/exp of unit i, the
            # rowsum/recip of unit i-1, and the PV/normalize of unit i-2,
            # so every unit's scalar chain hides behind later units' PE work
            for i in range(len(units) + 2):
                if i < len(units):
                    att_s(*units[i])
                if i == 1:
                    # hoist the gelu-set load into the attention phase
                    dmy = tinyp.tile([1, 1], F16, tag="tiny16")
                    nc.scalar.activation(dmy[:], ones16[0:1, 0:1], AF.Gelu)
                if 1 <= i < len(units) + 1:
                    att_norm(*units[i - 1])
                if 2 <= i:
                    att_out(*units[i - 2])

            # ---- proj + residual ----
            # proj split by image: img0's half streams on PE while img1's
            # softmax scalar chain is still finishing
            pj = [psB.tile([128, 3, T], F32, tag="psB", name="pj3")
                  for _ in range(2)]
            for b in range(BL):
                cs = slice(KT * b, KT * (b + 1))
                for grp in range(2):
                    for i in range(3):
                        oc = 3 * grp + i
                        for k in range(NCH):
                            nc.tensor.matmul(pj[grp][:, i, cs],
                                             wproj[k][:, 128 * oc:
                                                      128 * (oc + 1)],
                                             ot16[:, k, cs],
                                             start=(k == 0 and b == 0 and i == 0),
                                             stop=(k == NCH - 1 and b == BL - 1
                                                   and i == 2))
            Hn = hp.tile([128, NCH, T], F32, tag="H")
            lnin2 = lnp.tile([128, 2, NCH, T], F16, tag="lnin")
            for grp in range(2):
                drain_x16(pj[grp][:, :, :], grp, lnin2,
                          H[:, 3 * grp:3 * grp + 3, :])
            drain_H([pj[0][:, :, :], pj[1][:, :, :]], Hn,
                    [H[:, 0:3, :], H[:, 3:6, :]])
            H = Hn

            # ---- LN2 + MLP ----
            (mu16_2, mu32_2), anb16_2 = ln_stats(lnin2)
            g16 = gp.tile([128, MCH, T], F16, tag="g")
            ab2 = None
            for grp in range(MCH // 3):
                ps3 = psB.tile([128, 3, T], F32, tag="psB")
                for i in range(3):
                    oc = 3 * grp + i
                    for k in range(NCH):
                        h, col = divmod(128 * oc, 1536)
                        nc.tensor.matmul(ps3[:, i, :],
                                         wfc1[h][k][:, col:col + 128],
                                         lnin2[:, 0, k, :],
                                         start=(k == 0), stop=False)
                    row, coff = divmod(3 * D + 128 * oc, 2688)
                    mu_ap = mu16_2[0:1, :] if row == 0 else mu32_2[32:33, :]
                    nc.tensor.matmul(ps3[:, i, :],
                                     wvec_t[32 * row:32 * row + 1,
                                            coff:coff + 128],
                                     mu_ap, start=False, stop=True)
                if grp == 0:
                    ab2 = ln_bcast(anb16_2)
                nc.vector.tensor_mul(ps3[:, :, :], ps3[:, :, :],
                                     bfree(ab2[:, 0, :], 3))
                nc.scalar.activation(g16[:, 3 * grp:3 * (grp + 1), :], ps3[:, :, :],
                                     AF.Gelu)
            # hoist the exp-set load into the MLP phase (covers the next
            # layer's attention exps and this layer's trailing squares --
            # square lives in every set)
            dmy = tinyp.tile([1, 1], F16, tag="tiny16")
            nc.scalar.activation(dmy[:], ones16[0:1, 0:1], AF.Exp)
            # fc2 with k OUTER in halves so weight k-tiles die right after
            # use and next-layer DMA streams during this stage; two psB
            # accumulator tiles (oc 0-2 / 3-5) drain progressively.
            acc = [psB.tile([128, 3, T], F32, tag="psB", name="acc2")
                   for _ in range(2)]
            Hn = hp.tile([128, NCH, T], F32, tag="H")
            lnin_n = lnp.tile([128, 2, NCH, T], F16, tag="lnin")
            KH = MCH // 2
            for half in range(2):
                for oc in range(NCH):
                    for kk in range(KH):
                        k = half * KH + kk
                        nc.tensor.matmul(acc[oc // 3][:, oc % 3, :],
                                         wfc2_t[k // 6][:, k % 6,
                                                        128 * oc:128 * (oc + 1)],
                                         g16[:, k, :],
                                         start=(k == 0 and oc % 3 == 0),
                                         stop=(k == MCH - 1 and oc % 3 == 2))
                    if half == 1 and oc % 3 == 2:
                        bank = oc // 3
                        drain_x16(acc[bank][:, :, :], bank, lnin_n,
                                  H[:, 3 * bank:3 * bank + 3, :])
            drain_H([acc[0][:, :, :], acc[1][:, :, :]], Hn,
                    [H[:, 0:3, :], H[:, 3:6, :]])
            H = Hn
            lnin1 = lnin_n

        # ---- final LN (fp16 out; host upcasts) + store ----
        (mu16_f, mu32_f), anb16_f = ln_stats(lnin1)
        abf = ln_bcast(anb16_f)
        yf = otp.tile([128, NCH, T], F16, tag="ot", name="yf")
        for grp in range(2):
            sl = slice(3 * grp, 3 * (grp + 1))
            nc.vector.scalar_tensor_tensor(yf[:, sl, :], lnin1[:, 0, sl, :], 1.0,
                                           bfree(abf[:, 1, :], 3),
                                           op0=OP.mult, op1=OP.subtract)
            nc.vector.tensor_mul(yf[:, sl, :], yf[:, sl, :],
                                 bfree(abf[:, 0, :], 3))
            nc.sync.dma_start(out=out_d[:, sl, :], in_=yf[:, sl, :])

    nc.compile()
    return nc


def prep_inputs(inputs, depth=DEPTH):
    """Host-side marshalling. Returns per-core in_maps list."""
    g = {k: np.asarray(v) for k, v in inputs.items()}
    x = g["x"].astype(np.float32)
    noise = g["noise"].astype(np.float32)
    attn_mask = g["attn_mask"].astype(np.float32)
    ids_y = g["pos_embed_y_ids"].astype(np.int64)

    ids_shuffle = np.argsort(noise, axis=1, kind="stable")
    ids_keep = ids_shuffle[:, :LEN_KEEP]                      # (B, 75)

    patches = x.reshape(B, GH, GW, Q_).reshape(B, L, Q_)      # (B, 300, 100)
    mask_l = attn_mask.reshape(B, L)

    # pos vector per patch: [pos_y(384) | pos_x(384) * mask]
    pos_y = g["pos_y_table"].astype(np.float32)               # (13, 384)
    pos_x = g["pos_embed_x"].astype(np.float32)[0]            # (26, 384)
    ids_y_l = ids_y.reshape(B, L)
    gw_idx = np.tile(np.arange(GW), GH)                       # (300,)
    pos_full = np.zeros((B, L, D), np.float32)
    pos_full[:, :, :D // 2] = pos_y[ids_y_l]
    pos_full[:, :, D // 2:] = mask_l[:, :, None] * pos_x[gw_idx + 1][None]

    cls_vec = g["cls_token"].astype(np.float32).reshape(D).copy()
    cls_vec[D // 2:] += pos_x[0]

    wqkvT = np.ascontiguousarray(
        g["qkv_w"].astype(np.float32).transpose(0, 2, 1)[:depth]).astype(np.float16)
    wprojT = np.ascontiguousarray(
        g["proj_w"].astype(np.float32).transpose(0, 2, 1)[:depth]).astype(np.float16)
    wfc1T = np.ascontiguousarray(
        g["fc1_w"].astype(np.float32).transpose(0, 2, 1)[:depth]).astype(np.float16)
    wfc2T = np.ascontiguousarray(
        g["fc2_w"].astype(np.float32).transpose(0, 2, 1)[:depth]).astype(np.float16)
    wpatchT = np.ascontiguousarray(
        g["conv_w"].astype(np.float32).reshape(D, Q_).T).astype(np.float16)

    wsqn = -wqkvT.astype(np.float32).sum(axis=1).astype(np.float16)  # (depth, 3D)
    wsf1n = -wfc1T.astype(np.float32).sum(axis=1).astype(np.float16)
    wvecs = np.ascontiguousarray(np.concatenate([wsqn, wsf1n], axis=1)
                                 .reshape(depth, 2, 2688))

    in_maps = []
    for core in range(NCORES):
        patchesT = np.zeros((PIX, T), np.float16)
        posT = np.zeros((D, T), np.float32)
        mv = np.zeros((BL, KT), np.float16)
        for b in range(BL):
            img = core * BL + b
            sel = ids_keep[img]                               # (75,)
            patchesT[:, KT * b + 1:KT * (b + 1)] = patches[img, sel].T
            posT[:, KT * b] = cls_vec
            posT[:, KT * b + 1:KT * (b + 1)] = pos_full[img, sel].T
            mv[b, 0] = 1.0
            mv[b, 1:] = mask_l[img, np.sort(sel)]
        mvf = np.tile(mv.astype(np.float16)[:, None, :], (1, 12, 1)).reshape(
            1, BL, 2, 6 * KT)
        in_maps.append({
            "patchesT": patchesT,
            "posT": posT.reshape(NCH, 128, T),
            "mvec": mv,
            "mvecf": mvf,
            "wpatchT": wpatchT,
            "wqkvT": wqkvT,
            "wprojT": wprojT,
            "wfc1T": wfc1T,
            "wfc2T": wfc2T,
            "wvecs": wvecs,
        })
    return in_maps


_NC_CACHE = {}


def kernel(**inputs):
    if "nc" not in _NC_CACHE:
        _NC_CACHE["nc"] = build()
    nc = _NC_CACHE["nc"]
    in_maps = prep_inputs(inputs)
    res = run_bass_kernel_spmd(nc, in_maps, list(range(NCORES)))
    # device output is feature-major [p, c, t] with feature = 128*c + p
    outs = []
    for i in range(NCORES):
        a = res.results[i]["out"].reshape(128, NCH, T).astype(np.float32)
        a = a.transpose(1, 0, 2).reshape(D, T)
        outs.append(np.ascontiguousarray(a.T).reshape(BL, KT, D))
    return np.concatenate(outs, axis=0).astype(np.float32)


# revision 56
# speedup vs baseline: 1.0087x; 1.0087x over previous
"""MAE ViT encoder (nn_MaskedAutoencoderViT) Trainium2 Bass kernel.

Strategy: data-parallel over batch (16 images -> 8 cores x 2 images).
Feature-major activation layout on chip: activations stored transposed as
[128 partitions (d chunk), 6 chunks, 152 tokens] so every matmul is
weight-stationary (lhsT = 128x128 weight tile, rhs = activation columns)
with zero on-device transposes.  Attention is computed in transposed form
(S^T = (K^T)-stationary @ Q^T), softmax uses the structure
exp(att)/ (sum + 1e-9) (the reference's global-max subtraction cancels in
the normalization up to ~1e-10 relative, far below fp32 noise).
Matmul operands in fp16 (full PE rate, 11-bit mantissa), accumulation and
residual stream in fp32.

Scheduling structure (all serialization chains measured in the timeline
cost model):
- weight streaming: one DMA per k-chunk / fc2-quarter; DMA issue and the
  shared HWDGE device serialize per-instruction, so chunk-merged DMAs
  keep both off the critical path (transfer cost is bytes-based).
- each residual drain runs twice in parallel: Pool produces the fp32
  stream (H, double-buffered), DVE produces the fp16 matmul operand x16
  directly into the LN staging tile; ACT squares follow per chunk, so
  LN stats and the next GEMM both unblock ~0.6us after the psum stop.
- LN mean/rstd travel to all partitions via a 1x128 ones-matmul into
  PSUM (~130ns on the idle-ish PE) instead of gpsimd partition_broadcast
  (~1.4us on Pool).
- QKV and fc1 run on raw x16 with a K=1 colsum*mu correction row; V runs
  on raw x16 too, with the correction applied token-major and rstd folded
  into the per-token mask scale (needs one 1-col PE transpose).
- exp/gelu activation-table loads are hoisted off the critical path by
  dummy [1,1] activations issued while ACT is idle.
Host side does only data marshalling: noise argsort, patch gather,
pos-embed gathers, weight transposition + fp16 cast.
"""
import numpy as np
from contextlib import ExitStack

import concourse.bass as bass
import concourse.bacc as bacc
import concourse.mybir as mybir
import concourse.tile as tile
import bass_rust as _bass_rust
from concourse.bass_utils import run_bass_kernel_spmd
from concourse.hw_specs import get_activation_tables


class _Bacc(bacc.Bacc):
    """Bacc whose ACT-table-load pass prefers multi-function sets.

    The stock pass picks the first table set containing each activation
    function, which sends Ln to `natural_log` and Exp to `exp_and_others`
    and thrashes the table RAM inside every layernorm.  Reordering the
    set dict so `natural_log_exp_and_others` comes first makes Ln and Exp
    share one resident set (2 loads per layer total: exp-set <-> gelu-set).
    """

    def insert_act_table_loads(self):
        has_activation = any(
            isinstance(i, mybir.InstActivation)
            for b in self.main_func.blocks
            for i in b.instructions
        )
        if not has_activation:
            return
        # Keep the canonical set order (set ids are positional and the
        # runtime resolves them canonically) but hide Exp/Ln from every
        # other set so the chooser lands on the combined one.
        tabs = get_activation_tables(self.m.arch)
        items = []
        for k, v in tabs.items():
            if k != "natural_log_exp_and_others":
                v = {f for f in v if f.name not in ("Exp", "Ln")}
            items.append((k, v))
        _bass_rust.insert_act_table_loads(self, items)

F16 = mybir.dt.float16
F32 = mybir.dt.float32
AF = mybir.ActivationFunctionType
OP = mybir.AluOpType

# --- model config (hardcoded from the problem spec) ---
B, C_IN, H_IN, W_IN = 16, 1, 12, 2500
P_, Q_ = 1, 100
D, NH, DEPTH = 768, 12, 12
GH, GW = 12, 25
L = GH * GW                      # 300
LEN_KEEP = 75
HD = D // NH                     # 64
SCALE = HD ** -0.5               # 0.125
EPS_LN = 1e-5
MLP = 4 * D                      # 3072

NCORES = 8
BL = B // NCORES                 # 2 images per core
KT = 1 + LEN_KEEP                # 76 tokens per image
T = BL * KT                      # 152 token columns per core
NCH = D // 128                   # 6 feature chunks
MCH = MLP // 128                 # 24 mlp chunks
PIX = P_ * Q_                    # 100 pixels per patch


def bfree(ap, n, at=1):
    """Insert a 0-step (broadcast) free dim of size n at position `at`."""
    new_ap = list(ap.ap[:at]) + [[0, n]] + list(ap.ap[at:])
    return bass.AP(tensor=ap.tensor, offset=ap.offset, ap=new_ap)


def build(depth=DEPTH):
    nc = _Bacc("TRN2", target_bir_lowering=False, debug=False,
               num_devices=NCORES)

    # DRAM I/O
    patchesT = nc.dram_tensor("patchesT", [PIX, T], F16, kind="ExternalInput").ap()
    posT = nc.dram_tensor("posT", [NCH, 128, T], F32, kind="ExternalInput").ap()
    mvec = nc.dram_tensor("mvec", [BL, KT], F16, kind="ExternalInput").ap()
    mvecf = nc.dram_tensor("mvecf", [1, BL, 2, 6 * KT], F16, kind="ExternalInput").ap()
    wpatchT = nc.dram_tensor("wpatchT", [PIX, D], F16, kind="ExternalInput").ap()
    wqkvT = nc.dram_tensor("wqkvT", [depth, D, 3 * D], F16,
                           kind="ExternalInput").ap()
    wprojT = nc.dram_tensor("wprojT", [depth, D, D], F16,
                            kind="ExternalInput").ap()
    wfc1T = nc.dram_tensor("wfc1T", [depth, D, MLP], F16, kind="ExternalInput").ap()
    wfc2T = nc.dram_tensor("wfc2T", [depth, MLP, D], F16, kind="ExternalInput").ap()
    # [0:2D]: -colsum(Wq|Wk); [2D:3D]: -colsum(Wv); [3D:]: -colsum(Wfc1),
    # packed two partition rows of 2688 so the tile costs half the columns
    wvecs = nc.dram_tensor("wvecs", [depth, 2, 2688], F16,
                           kind="ExternalInput").ap()
    out_d = nc.dram_tensor("out", [128, NCH, T], F16, kind="ExternalOutput").ap()

    with tile.TileContext(nc) as tc, ExitStack() as ctx:
        pool = lambda name, bufs, **kw: ctx.enter_context(
            tc.tile_pool(name=name, bufs=bufs, **kw))

        const = pool("const", 1)
        hp = pool("hp", 2)
        lnp = pool("lnp", 2)
        qkp = pool("qkp", 1)
        vp = pool("vp", 2)
        ep = pool("ep", 2)
        otp = pool("otp", 1)
        gp = pool("gp", 1)
        tinyp = pool("tinyp", 5)
        medp = pool("medp", 3)
        bcp = pool("bcp", 2)
        wsump = pool("wsump", 2)
        wqkvp = pool("wqkvp", 7)
        wprojp = pool("wprojp", 7)
        wfc1p = pool("wfc1p", 14)
        wfc2p = pool("wfc2p", 6)

        psB = pool("psB", 3, space="PSUM")
        psC = pool("psC", 4, space="PSUM")
        pab = pool("pab", 1, space="PSUM")

        # constants
        ones16 = const.tile([128, 1], F16)
        nc.vector.memset(ones16[:], 1.0)
        onesr = const.tile([1, 64], F16)
        nc.vector.memset(onesr[:], 1.0)
        onesr128 = const.tile([1, 128], F16)
        nc.vector.memset(onesr128[:], 1.0)
        eps_t = const.tile([1, 1], F32)
        nc.vector.memset(eps_t[:], EPS_LN)

        # static inputs; the patch-embed-only tiles borrow weight-pool
        # slots (they die right after the patch embed, so the slots return
        # to the steady-state weight stream)
        patches_sb = wfc1p.tile([PIX, T], F16, tag="wfc1", name="patches_sb")
        nc.sync.dma_start(out=patches_sb[:], in_=patchesT[:])
        wpatch_sb = wfc1p.tile([PIX, D], F16, tag="wfc1", name="wpatch_sb")
        nc.sync.dma_start(out=wpatch_sb[:], in_=wpatchT[:])
        pos_sb = wfc2p.tile([128, NCH, T], F32, tag="wfc2", name="pos_sb")
        nc.sync.dma_start(out=pos_sb[:], in_=posT.rearrange("c p t -> p c t"))
        m_sb = const.tile([KT, BL], F16)
        nc.sync.dma_start(out=m_sb[:], in_=mvec.rearrange("b t -> t b"))
        m32_sb = const.tile([KT, BL], F32)
        nc.vector.tensor_copy(m32_sb[:], m_sb[:])
        mf_sb = const.tile([1, BL, 2, 6 * KT], F16)
        nc.sync.dma_start(out=mf_sb[:], in_=mvecf[:])

        def drain(ps_aps, h_old, h_new, lnin_new, add_sb=None):
            """Residual drain, both 3-chunk groups: DVE writes the fp16
            matmul copies into lnin_new[:,0] first (the critical
            successors), then squares the second group at fp16 2x rate
            while ACT squares the first; the fp32 stream updates into
            h_new go last (they are not needed until the next drain).
            Pool has no PSUM port on TRN2, so everything lives on DVE/ACT.
            add_sb: use these sbuf tensors instead of h_old (patch embed)."""
            sls = [slice(0, 3), slice(3, 6)]
            srcs = [add_sb[g] if add_sb is not None else h_old[:, sls[g], :]
                    for g in range(2)]
            for g in range(2):
                nc.vector.tensor_add(lnin_new[:, 0, sls[g], :], srcs[g],
                                     ps_aps[g])
                if g == 0:
                    nc.scalar.activation(lnin_new[:, 1, sls[0], :],
                                         lnin_new[:, 0, sls[0], :], AF.Square)
            nc.vector.tensor_mul(lnin_new[:, 1, sls[1], :],
                                 lnin_new[:, 0, sls[1], :],
                                 lnin_new[:, 0, sls[1], :])
            for g in range(2):
                nc.vector.tensor_add(h_new[:, sls[g], :], srcs[g], ps_aps[g])

        def ln_stats(lnin):
            """lnin [128,2,NCH,T] fp16 prefilled with [x | x^2].
            Returns (mu16 [1,T] f16, ab psum [128,2,T] f32 with
            [:,0]=rstd, [:,1]=mu broadcast, anb16 [1,2,T] f16)."""
            st = psC.tile([1, 2, T], F32, tag="psC", name="st")
            for c in range(NCH):
                nc.tensor.matmul(st[:], ones16[:, 0:1], lnin[:, :, c, :],
                                 start=(c == 0), stop=(c == NCH - 1))
            mean = tinyp.tile([1, T], F32, tag="tiny")
            nc.vector.tensor_scalar_mul(mean[:], st[0:1, 0, :], 1.0 / D)
            msq = tinyp.tile([1, T], F32, tag="tiny")
            nc.vector.tensor_mul(msq[:], mean[:], mean[:])
            v = tinyp.tile([1, T], F32, tag="tiny")
            nc.vector.scalar_tensor_tensor(v[:], st[0:1, 1, :], 1.0 / D, msq[:],
                                           op0=OP.mult, op1=OP.subtract)
            # rstd = exp(-0.5*ln(v+eps)) on ACT: Ln and Exp live in the
            # same table set (see _Bacc), and this replaces the 8-op DVE
            # Newton chain (each [1,T] op pays ~150ns of sem latency)
            anb16 = medp.tile([1, 2, T], F16, tag="anb")
            lnv = tinyp.tile([1, T], F32, tag="tiny")
            nc.scalar.activation(lnv[:], v[:], AF.Ln, bias=eps_t[0:1, 0:1])
            nc.scalar.activation(anb16[0:1, 0, :], lnv[:], AF.Exp, scale=-0.5)
            nc.vector.tensor_copy(anb16[0:1, 1, :], mean[:])
            # mu16 on two partition rows (the packed wvec correction rows
            # need a matching rhs base partition)
            # fast path row-0 copy feeds the early corrections; the
            # row-32 replica (packed-wvec fc1 slices) comes from a Pool
            # broadcast that never gates the LN chain
            mu16 = tinyp.tile([1, T], F16, tag="tiny16")
            nc.vector.tensor_copy(mu16[:], mean[:])
            mu32 = tinyp.tile([33, T], F16, tag="mu2")
            nc.gpsimd.partition_broadcast(mu32[:], mu16[:])
            return (mu16, mu32), anb16

        def ln_bcast(anb16):
            """Broadcast [1,2,T] across partitions via PE ones-matmul,
            then ACT-copy PSUM->SBUF (the drains' other operand is PSUM,
            and only one PSUM operand per DVE op is legal)."""
            ab = pab.tile([128, 2, T], F32, tag="pab")
            nc.tensor.matmul(ab[:], onesr128[:], anb16[:],
                             start=True, stop=True)
            ab_sb = bcp.tile([128, 2, T], F16, tag="bc")
            nc.scalar.activation(ab_sb[:], ab[:], AF.Copy)
            return ab_sb

        # residual stream, feature-major fp32 (double-buffered so the
        # Pool-side update overlaps the DVE-side fp16 cast)
        H = hp.tile([128, NCH, T], F32, tag="H")
        lnin1 = lnp.tile([128, 2, NCH, T], F16, tag="lnin")

        # ---- patch embed + pos add ----
        pe_ps = []
        for grp in range(2):
            ps3 = psB.tile([128, 3, T], F32, tag="psB", name="pe3")
            for i in range(3):
                c = 3 * grp + i
                nc.tensor.matmul(ps3[:, i, :], wpatch_sb[:, 128 * c:128 * (c + 1)],
                                 patches_sb[:], start=(i == 0), stop=(i == 2))
            pe_ps.append(ps3)
        drain([p[:, :, :] for p in pe_ps], None, H, lnin1,
              add_sb=[pos_sb[:, 0:3, :], pos_sb[:, 3:6, :]])

        for l in range(depth):
            # weight loads for this layer (emitted first so DMA starts early)
            wvec_t = wsump.tile([33, 2688], F16, tag="wvec")
            nc.sync.dma_start(out=wvec_t[0:33:32, :], in_=wvecs[l])
            # DMA queue is FIFO; emit in expected buffer-free order so a
            # gate-blocked load never shadows an already-loadable one:
            # spare-slot loads first, then by when the prior layer's tiles
            # die (qkv ~35%, proj ~52%, fc1a ~62%, fc1b ~75%, fc2 late).
            wqkv = [wqkvp.tile([128, 3 * D], F16, tag="wqkv", name="wqkv")
                    for _ in range(NCH)]
            wfc1 = [[wfc1p.tile([128, MLP // 2], F16, tag="wfc1", name="wfc1")
                     for _ in range(NCH)] for _ in range(2)]
            wproj = [wprojp.tile([128, D], F16, tag="wproj", name="wproj")
                     for _ in range(NCH)]
            wfc2_t = [wfc2p.tile([128, 6, D], F16, tag="wfc2", name="wfc2")
                      for _ in range(4)]

            def dma_qkv(k):
                nc.sync.dma_start(out=wqkv[k][:],
                                  in_=wqkvT[l, 128 * k:128 * (k + 1), :])

            def dma_fc1(h, k):
                nc.sync.dma_start(out=wfc1[h][k][:],
                                  in_=wfc1T[l, 128 * k:128 * (k + 1),
                                            1536 * h:1536 * (h + 1)])

            def dma_proj(k):
                nc.sync.dma_start(out=wproj[k][:],
                                  in_=wprojT[l, 128 * k:128 * (k + 1), :])

            def dma_fc2(q):
                nc.sync.dma_start(
                    out=wfc2_t[q][:],
                    in_=wfc2T[l, 768 * q:768 * (q + 1), :].rearrange(
                        "(k p) j -> p k j", p=128))

            for k in range(NCH):
                dma_qkv(k)
            dma_fc1(0, 0)
            dma_fc1(0, 1)
            dma_fc2(0)
            dma_fc2(1)
            for k in range(NCH):
                dma_proj(k)
            for k in range(2, NCH):
                dma_fc1(0, k)
            for k in range(NCH):
                dma_fc1(1, k)
            dma_fc2(2)
            dma_fc2(3)

            # ---- LN1 stats (x and x^2 already staged in lnin1) ----
            (mu16_1, mu32_1), anb16_1 = ln_stats(lnin1)

            # ---- QKV: Q,K feature-major on raw x16 + K=1 correction ----
            qk16 = qkp.tile([128, 2 * NCH, T], F16, tag="qk")
            ab1 = None
            first = True
            for grp in [0, 2, 1, 3]:
                ps3 = psB.tile([128, 3, T], F32, tag="psB", name="qk3")
                for i in range(3):
                    oc = 3 * grp + i
                    for k in range(NCH):
                        nc.tensor.matmul(ps3[:, i, :],
                                         wqkv[k][:, 128 * oc:128 * (oc + 1)],
                                         lnin1[:, 0, k, :],
                                         start=(k == 0), stop=False)
                    nc.tensor.matmul(ps3[:, i, :],
                                     wvec_t[0:1, 128 * oc:128 * (oc + 1)],
                                     mu16_1[:], start=False, stop=True)
                if first:
                    # emitted after the first group's matmuls: the PE sits
                    # behind them in queue order, so the wait on the DVE
                    # stats chain overlaps that GEMM stream
                    ab1 = ln_bcast(anb16_1)
                    first = False
                nc.vector.tensor_mul(qk16[:, 3 * grp:3 * (grp + 1), :],
                                     ps3[:, :, :], bfree(ab1[:, 0, :], 3))

            # per-image rstd as a column (PE transpose) * mask -> V scale
            csc = []
            for b in range(BL):
                vt = psC.tile([KT, 1], F16, tag="psC", name="vt")
                nc.tensor.matmul(vt[:], anb16_1[0:1, 0, KT * b:KT * (b + 1)],
                                 ones16[0:1, 0:1], is_transpose=True)
                cs = tinyp.tile([KT, 1], F32, tag="csc")
                nc.vector.tensor_mul(cs[:], vt[:], m32_sb[:, b:b + 1])
                csc.append(cs)

            # ---- V token-major per image, raw x16 + correction ----
            v16 = []
            for b in range(BL):
                vps0 = psC.tile([KT, 384], F32, tag="psC", name="vps")
                vps1 = psC.tile([KT, 384], F32, tag="psC", name="vps")
                for k in range(NCH):
                    nc.tensor.matmul(vps0[:],
                                     lnin1[:, 0, k, KT * b:KT * (b + 1)],
                                     wqkv[k][:, 2 * D:2 * D + 384],
                                     start=(k == 0), stop=False)
                nc.tensor.matmul(vps0[:], mu16_1[0:1, KT * b:KT * (b + 1)],
                                 wvec_t[0:1, 2 * D:2 * D + 384],
                                 start=False, stop=True)
                for k in range(NCH):
                    nc.tensor.matmul(vps1[:],
                                     lnin1[:, 0, k, KT * b:KT * (b + 1)],
                                     wqkv[k][:, 2 * D + 384:3 * D],
                                     start=(k == 0), stop=False)
                nc.tensor.matmul(vps1[:], mu16_1[0:1, KT * b:KT * (b + 1)],
                                 wvec_t[0:1, 2 * D + 384:3 * D],
                                 start=False, stop=True)
                v = vp.tile([KT, D], F16, tag="v")
                nc.vector.tensor_scalar_mul(v[:, 0:384], vps0[:], csc[b][:])
                nc.vector.tensor_scalar_mul(v[:, 384:768], vps1[:], csc[b][:])
                v16.append(v)

            # ---- attention; images interleaved, heads grouped by parity.
            # PV runs on the raw exp(S) values: the attn mask is folded into
            # the V drain (row scale) and the 1/rowsum normalization into the
            # ot16 assembly multiply, so the softmax scalar chain never
            # blocks the PE stream.
            # fully pipelined per-(image, head-group) units: each unit's
            # exp->rowsum->recip->mask->bcast->assembly chain hides behind
            # the later units' S/PV matmul stream.  No softmax eps: rowsums
            # of exp() are bounded >= e^-O(1) here, and the reference's
            # +1e-9 shifts them by ~1e-11 relative.
            ot16 = otp.tile([128, NCH, T], F16, tag="ot")
            e16s = [ep.tile([KT, 2, 6 * KT], F16, tag="e", name="e16")
                    for _ in range(BL)]
            rrs = [medp.tile([1, 2, 6 * KT], F16, tag="rr", name="rr")
                   for _ in range(BL)]
            rrbs = [bcp.tile([64, 2, 6 * KT], F16, tag="rb", name="rrb")
                    for _ in range(BL)]
            units = [(b, g) for b in range(BL) for g in range(2)]

            def att_s(b, g):
                sps = psC.tile([KT, 6 * KT], F32, tag="psC", name="sps")
                for j in range(6):
                    nc.tensor.matmul(
                        sps[:, KT * j:KT * (j + 1)],
                        qk16[64 * g:64 * (g + 1), 6 + j, KT * b:KT * (b + 1)],
                        qk16[64 * g:64 * (g + 1), j, KT * b:KT * (b + 1)],
                        start=True, stop=True)
                nc.scalar.activation(e16s[b][:, g, :], sps[:],
                                     AF.Exp, scale=SCALE)

            def att_norm(b, g):
                rps = psC.tile([1, 6 * KT], F32, tag="psC", name="rps")
                nc.tensor.matmul(rps[:], m_sb[:, b:b + 1],
                                 e16s[b][:, g, :], start=True, stop=True)
                with nc.allow_low_precision(reason="softmax norm fp16"):
                    nc.vector.reciprocal(rrs[b][0:1, g, :], rps[:])

            def att_out(b, g):
                ops = psC.tile([64, 6 * KT], F32, tag="psC", name="ops")
                for j in range(6):
                    nc.tensor.matmul(
                        ops[:, KT * j:KT * (j + 1)],
                        v16[b][:, 128 * j + 64 * g:128 * j + 64 * g + 64],
                        e16s[b][:, g, KT * j:KT * (j + 1)],
                        start=True, stop=True)
                rbp = psC.tile([64, 6 * KT], F32, tag="psC", name="rbp")
                nc.tensor.matmul(rbp[:], onesr[:], rrs[b][0:1, g, :],
                                 start=True, stop=True)
                nc.scalar.activation(rrbs[b][:, g, :], rbp[:], AF.Copy)
                nc.vector.tensor_mul(
                    ot16[64 * g:64 * (g + 1), :, KT * b:KT * (b + 1)],
                    ops[:].rearrange("p (j t) -> p j t", j=6),
                    rrbs[b][:, g, :].rearrange("p (j t) -> p j t", j=6))

            # 3-stage software pipeline: slot i runs S/exp of unit i, the
            # rowsum/recip of unit i-1, and the PV/normalize of unit i-2,
            # so every unit's scalar chain hides behind later units' PE work
            for i in range(len(units) + 2):
                if i < len(units):
                    att_s(*units[i])
                if i == 1:
                    # hoist the gelu-set load into the attention phase
                    dmy = tinyp.tile([1, 1], F16, tag="tiny16")
                    nc.scalar.activation(dmy[:], ones16[0:1, 0:1], AF.Gelu)
                if 1 <= i < len(units) + 1:
                    att_norm(*units[i - 1])
                if 2 <= i:
                    att_out(*units[i - 2])

            # ---- proj + residual ----
            # proj split by image: img0's half streams on PE while img1's
            # softmax scalar chain is still finishing
            pj = [psB.tile([128, 3, T], F32, tag="psB", name="pj3")
                  for _ in range(2)]
            for b in range(BL):
                cs = slice(KT * b, KT * (b + 1))
                for grp in range(2):
                    for i in range(3):
                        oc = 3 * grp + i
                        for k in range(NCH):
                            nc.tensor.matmul(pj[grp][:, i, cs],
                                             wproj[k][:, 128 * oc:
                                                      128 * (oc + 1)],
                                             ot16[:, k, cs],
                                             start=(k == 0 and b == 0 and i == 0),
                                             stop=(k == NCH - 1 and b == BL - 1
                                                   and i == 2))
            Hn = hp.tile([128, NCH, T], F32, tag="H")
            lnin2 = lnp.tile([128, 2, NCH, T], F16, tag="lnin")
            drain([pj[0][:, :, :], pj[1][:, :, :]], H, Hn, lnin2)
            H = Hn

            # ---- LN2 + MLP ----
            (mu16_2, mu32_2), anb16_2 = ln_stats(lnin2)
            g16 = gp.tile([128, MCH, T], F16, tag="g")
            ab2 = None
            for grp in range(MCH // 3):
                ps3 = psB.tile([128, 3, T], F32, tag="psB")
                for i in range(3):
                    oc = 3 * grp + i
                    for k in range(NCH):
                        h, col = divmod(128 * oc, 1536)
                        nc.tensor.matmul(ps3[:, i, :],
                                         wfc1[h][k][:, col:col + 128],
                                         lnin2[:, 0, k, :],
                                         start=(k == 0), stop=False)
                    row, coff = divmod(3 * D + 128 * oc, 2688)
                    mu_ap = mu16_2[0:1, :] if row == 0 else mu32_2[32:33, :]
                    nc.tensor.matmul(ps3[:, i, :],
                                     wvec_t[32 * row:32 * row + 1,
                                            coff:coff + 128],
                                     mu_ap, start=False, stop=True)
                if grp == 0:
                    ab2 = ln_bcast(anb16_2)
                nc.vector.tensor_mul(ps3[:, :, :], ps3[:, :, :],
                                     bfree(ab2[:, 0, :], 3))
                nc.scalar.activation(g16[:, 3 * grp:3 * (grp + 1), :], ps3[:, :, :],
                                     AF.Gelu)
            # hoist the exp-set load into the MLP phase (covers the next
            # layer's attention exps and this layer's trailing squares --
            # square lives in every set)
            dmy = tinyp.tile([1, 1], F16, tag="tiny16")
            nc.scalar.activation(dmy[:], ones16[0:1, 0:1], AF.Exp)
            # fc2 with k OUTER in halves so weight k-tiles die right after
            # use and next-layer DMA streams during this stage; two psB
            # accumulator tiles (oc 0-2 / 3-5) drain progressively.
            acc = [psB.tile([128, 3, T], F32, tag="psB", name="acc2")
                   for _ in range(2)]
            Hn = hp.tile([128, NCH, T], F32, tag="H")
            lnin_n = lnp.tile([128, 2, NCH, T], F16, tag="lnin")
            KH = MCH // 2
            for half in range(2):
                for oc in range(NCH):
                    for kk in range(KH):
                        k = half * KH + kk
                        nc.tensor.matmul(acc[oc // 3][:, oc % 3, :],
                                         wfc2_t[k // 6][:, k % 6,
                                                        128 * oc:128 * (oc + 1)],
                                         g16[:, k, :],
                                         start=(k == 0 and oc % 3 == 0),
                                         stop=(k == MCH - 1 and oc % 3 == 2))
                    if half == 1 and oc == 2:
                        # bank A closed: start its fp16 cast + ACT square
                        # while bank B still accumulates on PE
                        nc.vector.tensor_add(lnin_n[:, 0, 0:3, :],
                                             H[:, 0:3, :], acc[0][:, :, :])
                        nc.scalar.activation(lnin_n[:, 1, 0:3, :],
                                             lnin_n[:, 0, 0:3, :], AF.Square)
            nc.vector.tensor_add(lnin_n[:, 0, 3:6, :], H[:, 3:6, :],
                                 acc[1][:, :, :])
            nc.vector.tensor_mul(lnin_n[:, 1, 3:6, :], lnin_n[:, 0, 3:6, :],
                                 lnin_n[:, 0, 3:6, :])
            for g in range(2):
                nc.vector.tensor_add(Hn[:, 3 * g:3 * g + 3, :],
                                     H[:, 3 * g:3 * g + 3, :],
                                     acc[g][:, :, :])
            H = Hn
            lnin1 = lnin_n

        # ---- final LN (fp16 out; host upcasts) + store ----
        (mu16_f, mu32_f), anb16_f = ln_stats(lnin1)
        abf = ln_bcast(anb16_f)
        yf = otp.tile([128, NCH, T], F16, tag="ot", name="yf")
        for grp in range(2):
            sl = slice(3 * grp, 3 * (grp + 1))
            nc.vector.scalar_tensor_tensor(yf[:, sl, :], lnin1[:, 0, sl, :], 1.0,
                                           bfree(abf[:, 1, :], 3),
                                           op0=OP.mult, op1=OP.subtract)
            nc.vector.tensor_mul(yf[:, sl, :], yf[:, sl, :],
                                 bfree(abf[:, 0, :], 3))
            nc.sync.dma_start(out=out_d[:, sl, :], in_=yf[:, sl, :])

    nc.compile()
    return nc


def prep_inputs(inputs, depth=DEPTH):
    """Host-side marshalling. Returns per-core in_maps list."""
    g = {k: np.asarray(v) for k, v in inputs.items()}
    x = g["x"].astype(np.float32)
    noise = g["noise"].astype(np.float32)
    attn_mask = g["attn_mask"].astype(np.float32)
    ids_y = g["pos_embed_y_ids"].astype(np.int64)

    ids_shuffle = np.argsort(noise, axis=1, kind="stable")
    ids_keep = ids_shuffle[:, :LEN_KEEP]                      # (B, 75)

    patches = x.reshape(B, GH, GW, Q_).reshape(B, L, Q_)      # (B, 300, 100)
    mask_l = attn_mask.reshape(B, L)

    # pos vector per patch: [pos_y(384) | pos_x(384) * mask]
    pos_y = g["pos_y_table"].astype(np.float32)               # (13, 384)
    pos_x = g["pos_embed_x"].astype(np.float32)[0]            # (26, 384)
    ids_y_l = ids_y.reshape(B, L)
    gw_idx = np.tile(np.arange(GW), GH)                       # (300,)
    pos_full = np.zeros((B, L, D), np.float32)
    pos_full[:, :, :D // 2] = pos_y[ids_y_l]
    pos_full[:, :, D // 2:] = mask_l[:, :, None] * pos_x[gw_idx + 1][None]

    cls_vec = g["cls_token"].astype(np.float32).reshape(D).copy()
    cls_vec[D // 2:] += pos_x[0]

    wqkvT = np.ascontiguousarray(
        g["qkv_w"].astype(np.float32).transpose(0, 2, 1)[:depth]).astype(np.float16)
    wprojT = np.ascontiguousarray(
        g["proj_w"].astype(np.float32).transpose(0, 2, 1)[:depth]).astype(np.float16)
    wfc1T = np.ascontiguousarray(
        g["fc1_w"].astype(np.float32).transpose(0, 2, 1)[:depth]).astype(np.float16)
    wfc2T = np.ascontiguousarray(
        g["fc2_w"].astype(np.float32).transpose(0, 2, 1)[:depth]).astype(np.float16)
    wpatchT = np.ascontiguousarray(
        g["conv_w"].astype(np.float32).reshape(D, Q_).T).astype(np.float16)

    wsqn = -wqkvT.astype(np.float32).sum(axis=1).astype(np.float16)  # (depth, 3D)
    wsf1n = -wfc1T.astype(np.float32).sum(axis=1).astype(np.float16)
    wvecs = np.ascontiguousarray(np.concatenate([wsqn, wsf1n], axis=1)
                                 .reshape(depth, 2, 2688))

    in_maps = []
    for core in range(NCORES):
        patchesT = np.zeros((PIX, T), np.float16)
        posT = np.zeros((D, T), np.float32)
        mv = np.zeros((BL, KT), np.float16)
        for b in range(BL):
            img = core * BL + b
            sel = ids_keep[img]                               # (75,)
            patchesT[:, KT * b + 1:KT * (b + 1)] = patches[img, sel].T
            posT[:, KT * b] = cls_vec
            posT[:, KT * b + 1:KT * (b + 1)] = pos_full[img, sel].T
            mv[b, 0] = 1.0
            mv[b, 1:] = mask_l[img, np.sort(sel)]
        mvf = np.tile(mv.astype(np.float16)[:, None, :], (1, 12, 1)).reshape(
            1, BL, 2, 6 * KT)
        in_maps.append({
            "patchesT": patchesT,
            "posT": posT.reshape(NCH, 128, T),
            "mvec": mv,
            "mvecf": mvf,
            "wpatchT": wpatchT,
            "wqkvT": wqkvT,
            "wprojT": wprojT,
            "wfc1T": wfc1T,
            "wfc2T": wfc2T,
            "wvecs": wvecs,
        })
    return in_maps


_NC_CACHE = {}


def kernel(**inputs):
    if "nc" not in _NC_CACHE:
        _NC_CACHE["nc"] = build()
    nc = _NC_CACHE["nc"]
    in_maps = prep_inputs(inputs)
    res = run_bass_kernel_spmd(nc, in_maps, list(range(NCORES)))
    # device output is feature-major [p, c, t] with feature = 128*c + p
    outs = []
    for i in range(NCORES):
        a = res.results[i]["out"].reshape(128, NCH, T).astype(np.float32)
        a = a.transpose(1, 0, 2).reshape(D, T)
        outs.append(np.ascontiguousarray(a.T).reshape(BL, KT, D))
    return np.concatenate(outs, axis=0).astype(np.float32)


# revision 57
# speedup vs baseline: 1.0107x; 1.0019x over previous
"""MAE ViT encoder (nn_MaskedAutoencoderViT) Trainium2 Bass kernel.

Strategy: data-parallel over batch (16 images -> 8 cores x 2 images).
Feature-major activation layout on chip: activations stored transposed as
[128 partitions (d chunk), 6 chunks, 152 tokens] so every matmul is
weight-stationary (lhsT = 128x128 weight tile, rhs = activation columns)
with zero on-device transposes.  Attention is computed in transposed form
(S^T = (K^T)-stationary @ Q^T), softmax uses the structure
exp(att)/ (sum + 1e-9) (the reference's global-max subtraction cancels in
the normalization up to ~1e-10 relative, far below fp32 noise).
Matmul operands in fp16 (full PE rate, 11-bit mantissa), accumulation and
residual stream in fp32.

Scheduling structure (all serialization chains measured in the timeline
cost model):
- weight streaming: one DMA per k-chunk / fc2-quarter; DMA issue and the
  shared HWDGE device serialize per-instruction, so chunk-merged DMAs
  keep both off the critical path (transfer cost is bytes-based).
- each residual drain runs twice in parallel: Pool produces the fp32
  stream (H, double-buffered), DVE produces the fp16 matmul operand x16
  directly into the LN staging tile; ACT squares follow per chunk, so
  LN stats and the next GEMM both unblock ~0.6us after the psum stop.
- LN mean/rstd travel to all partitions via a 1x128 ones-matmul into
  PSUM (~130ns on the idle-ish PE) instead of gpsimd partition_broadcast
  (~1.4us on Pool).
- QKV and fc1 run on raw x16 with a K=1 colsum*mu correction row; V runs
  on raw x16 too, with the correction applied token-major and rstd folded
  into the per-token mask scale (needs one 1-col PE transpose).
- exp/gelu activation-table loads are hoisted off the critical path by
  dummy [1,1] activations issued while ACT is idle.
Host side does only data marshalling: noise argsort, patch gather,
pos-embed gathers, weight transposition + fp16 cast.
"""
import numpy as np
from contextlib import ExitStack

import concourse.bass as bass
import concourse.bacc as bacc
import concourse.mybir as mybir
import concourse.tile as tile
import bass_rust as _bass_rust
from concourse.bass_utils import run_bass_kernel_spmd
from concourse.hw_specs import get_activation_tables


class _Bacc(bacc.Bacc):
    """Bacc whose ACT-table-load pass prefers multi-function sets.

    The stock pass picks the first table set containing each activation
    function, which sends Ln to `natural_log` and Exp to `exp_and_others`
    and thrashes the table RAM inside every layernorm.  Reordering the
    set dict so `natural_log_exp_and_others` comes first makes Ln and Exp
    share one resident set (2 loads per layer total: exp-set <-> gelu-set).
    """

    def insert_act_table_loads(self):
        has_activation = any(
            isinstance(i, mybir.InstActivation)
            for b in self.main_func.blocks
            for i in b.instructions
        )
        if not has_activation:
            return
        # Keep the canonical set order (set ids are positional and the
        # runtime resolves them canonically) but hide Exp/Ln from every
        # other set so the chooser lands on the combined one.
        tabs = get_activation_tables(self.m.arch)
        items = []
        for k, v in tabs.items():
            if k != "natural_log_exp_and_others":
                v = {f for f in v if f.name not in ("Exp", "Ln")}
            items.append((k, v))
        _bass_rust.insert_act_table_loads(self, items)

F16 = mybir.dt.float16
F32 = mybir.dt.float32
AF = mybir.ActivationFunctionType
OP = mybir.AluOpType

# --- model config (hardcoded from the problem spec) ---
B, C_IN, H_IN, W_IN = 16, 1, 12, 2500
P_, Q_ = 1, 100
D, NH, DEPTH = 768, 12, 12
GH, GW = 12, 25
L = GH * GW                      # 300
LEN_KEEP = 75
HD = D // NH                     # 64
SCALE = HD ** -0.5               # 0.125
EPS_LN = 1e-5
MLP = 4 * D                      # 3072

NCORES = 8
BL = B // NCORES                 # 2 images per core
KT = 1 + LEN_KEEP                # 76 tokens per image
T = BL * KT                      # 152 token columns per core
NCH = D // 128                   # 6 feature chunks
MCH = MLP // 128                 # 24 mlp chunks
PIX = P_ * Q_                    # 100 pixels per patch


def bfree(ap, n, at=1):
    """Insert a 0-step (broadcast) free dim of size n at position `at`."""
    new_ap = list(ap.ap[:at]) + [[0, n]] + list(ap.ap[at:])
    return bass.AP(tensor=ap.tensor, offset=ap.offset, ap=new_ap)


def build(depth=DEPTH):
    nc = _Bacc("TRN2", target_bir_lowering=False, debug=False,
               num_devices=NCORES)

    # DRAM I/O
    patchesT = nc.dram_tensor("patchesT", [PIX, T], F16, kind="ExternalInput").ap()
    posT = nc.dram_tensor("posT", [NCH, 128, T], F32, kind="ExternalInput").ap()
    mvec = nc.dram_tensor("mvec", [BL, KT], F16, kind="ExternalInput").ap()
    mvecf = nc.dram_tensor("mvecf", [1, BL, 2, 6 * KT], F16, kind="ExternalInput").ap()
    wpatchT = nc.dram_tensor("wpatchT", [PIX, D], F16, kind="ExternalInput").ap()
    wqkvT = nc.dram_tensor("wqkvT", [depth, D, 3 * D], F16,
                           kind="ExternalInput").ap()
    wprojT = nc.dram_tensor("wprojT", [depth, D, D], F16,
                            kind="ExternalInput").ap()
    wfc1T = nc.dram_tensor("wfc1T", [depth, D, MLP], F16, kind="ExternalInput").ap()
    wfc2T = nc.dram_tensor("wfc2T", [depth, MLP, D], F16, kind="ExternalInput").ap()
    # [0:2D]: -colsum(Wq|Wk); [2D:3D]: -colsum(Wv); [3D:]: -colsum(Wfc1),
    # packed two partition rows of 2688 so the tile costs half the columns
    wvecs = nc.dram_tensor("wvecs", [depth, 2, 2688], F16,
                           kind="ExternalInput").ap()
    out_d = nc.dram_tensor("out", [128, NCH, T], F16, kind="ExternalOutput").ap()

    with tile.TileContext(nc) as tc, ExitStack() as ctx:
        pool = lambda name, bufs, **kw: ctx.enter_context(
            tc.tile_pool(name=name, bufs=bufs, **kw))

        const = pool("const", 1)
        hp = pool("hp", 2)
        lnp = pool("lnp", 2)
        qkp = pool("qkp", 1)
        vp = pool("vp", 2)
        ep = pool("ep", 2)
        otp = pool("otp", 1)
        gp = pool("gp", 1)
        tinyp = pool("tinyp", 5)
        medp = pool("medp", 3)
        bcp = pool("bcp", 2)
        wsump = pool("wsump", 2)
        wqkvp = pool("wqkvp", 7)
        wprojp = pool("wprojp", 7)
        wfc1p = pool("wfc1p", 14)
        wfc2p = pool("wfc2p", 6)

        psB = pool("psB", 3, space="PSUM")
        psC = pool("psC", 4, space="PSUM")
        pab = pool("pab", 1, space="PSUM")

        # constants
        ones16 = const.tile([128, 1], F16)
        nc.vector.memset(ones16[:], 1.0)
        onesr = const.tile([1, 64], F16)
        nc.vector.memset(onesr[:], 1.0)
        onesr128 = const.tile([1, 128], F16)
        nc.vector.memset(onesr128[:], 1.0)
        eps_t = const.tile([1, 1], F32)
        nc.vector.memset(eps_t[:], EPS_LN)

        # static inputs; the patch-embed-only tiles borrow weight-pool
        # slots (they die right after the patch embed, so the slots return
        # to the steady-state weight stream)
        patches_sb = wfc1p.tile([PIX, T], F16, tag="wfc1", name="patches_sb")
        nc.sync.dma_start(out=patches_sb[:], in_=patchesT[:])
        wpatch_sb = wfc1p.tile([PIX, D], F16, tag="wfc1", name="wpatch_sb")
        nc.sync.dma_start(out=wpatch_sb[:], in_=wpatchT[:])
        pos_sb = wfc2p.tile([128, NCH, T], F32, tag="wfc2", name="pos_sb")
        nc.sync.dma_start(out=pos_sb[:], in_=posT.rearrange("c p t -> p c t"))
        m_sb = const.tile([KT, BL], F16)
        nc.sync.dma_start(out=m_sb[:], in_=mvec.rearrange("b t -> t b"))
        m32_sb = const.tile([KT, BL], F32)
        nc.vector.tensor_copy(m32_sb[:], m_sb[:])
        mf_sb = const.tile([1, BL, 2, 6 * KT], F16)
        nc.sync.dma_start(out=mf_sb[:], in_=mvecf[:])

        def drain(ps_aps, h_old, h_new, lnin_new, add_sb=None):
            """Residual drain, both 3-chunk groups: DVE writes the fp16
            matmul copies into lnin_new[:,0] first (the critical
            successors), then squares the second group at fp16 2x rate
            while ACT squares the first; the fp32 stream updates into
            h_new go last (they are not needed until the next drain).
            Pool has no PSUM port on TRN2, so everything lives on DVE/ACT.
            add_sb: use these sbuf tensors instead of h_old (patch embed)."""
            sls = [slice(0, 3), slice(3, 6)]
            srcs = [add_sb[g] if add_sb is not None else h_old[:, sls[g], :]
                    for g in range(2)]
            for g in range(2):
                nc.vector.tensor_add(lnin_new[:, 0, sls[g], :], srcs[g],
                                     ps_aps[g])
                if g == 0:
                    nc.scalar.activation(lnin_new[:, 1, sls[0], :],
                                         lnin_new[:, 0, sls[0], :], AF.Square)
            nc.vector.tensor_mul(lnin_new[:, 1, sls[1], :],
                                 lnin_new[:, 0, sls[1], :],
                                 lnin_new[:, 0, sls[1], :])
            for g in range(2):
                nc.vector.tensor_add(h_new[:, sls[g], :], srcs[g], ps_aps[g])

        def ln_stats(lnin):
            """lnin [128,2,NCH,T] fp16 prefilled with [x | x^2].
            Returns (mu16 [1,T] f16, ab psum [128,2,T] f32 with
            [:,0]=rstd, [:,1]=mu broadcast, anb16 [1,2,T] f16)."""
            st = psC.tile([1, 2, T], F32, tag="psC", name="st")
            for c in range(NCH):
                nc.tensor.matmul(st[:], ones16[:, 0:1], lnin[:, :, c, :],
                                 start=(c == 0), stop=(c == NCH - 1))
            mean = tinyp.tile([1, T], F32, tag="tiny")
            nc.vector.tensor_scalar_mul(mean[:], st[0:1, 0, :], 1.0 / D)
            msq = tinyp.tile([1, T], F32, tag="tiny")
            nc.vector.tensor_mul(msq[:], mean[:], mean[:])
            v = tinyp.tile([1, T], F32, tag="tiny")
            nc.vector.scalar_tensor_tensor(v[:], st[0:1, 1, :], 1.0 / D, msq[:],
                                           op0=OP.mult, op1=OP.subtract)
            # rstd = exp(-0.5*ln(v+eps)) on ACT: Ln and Exp live in the
            # same table set (see _Bacc), and this replaces the 8-op DVE
            # Newton chain (each [1,T] op pays ~150ns of sem latency)
            anb16 = medp.tile([1, 2, T], F16, tag="anb")
            lnv = tinyp.tile([1, T], F32, tag="tiny")
            nc.scalar.activation(lnv[:], v[:], AF.Ln, bias=eps_t[0:1, 0:1])
            nc.scalar.activation(anb16[0:1, 0, :], lnv[:], AF.Exp, scale=-0.5)
            nc.vector.tensor_copy(anb16[0:1, 1, :], mean[:])
            # mu16 on two partition rows (the packed wvec correction rows
            # need a matching rhs base partition)
            # fast path row-0 copy feeds the early corrections; the
            # row-32 replica (packed-wvec fc1 slices) comes from a Pool
            # broadcast that never gates the LN chain
            mu16 = tinyp.tile([1, T], F16, tag="tiny16")
            nc.vector.tensor_copy(mu16[:], mean[:])
            mu32 = tinyp.tile([33, T], F16, tag="mu2")
            nc.gpsimd.partition_broadcast(mu32[:], mu16[:])
            return (mu16, mu32), anb16

        def ln_bcast(anb16):
            """Broadcast [1,2,T] across partitions via PE ones-matmul,
            then ACT-copy PSUM->SBUF (the drains' other operand is PSUM,
            and only one PSUM operand per DVE op is legal)."""
            ab = pab.tile([128, 2, T], F32, tag="pab")
            nc.tensor.matmul(ab[:], onesr128[:], anb16[:],
                             start=True, stop=True)
            ab_sb = bcp.tile([128, 2, T], F16, tag="bc")
            nc.scalar.activation(ab_sb[:], ab[:], AF.Copy)
            return ab_sb

        # residual stream, feature-major fp32 (double-buffered so the
        # Pool-side update overlaps the DVE-side fp16 cast)
        H = hp.tile([128, NCH, T], F32, tag="H")
        lnin1 = lnp.tile([128, 2, NCH, T], F16, tag="lnin")

        # ---- patch embed + pos add ----
        pe_ps = []
        for grp in range(2):
            ps3 = psB.tile([128, 3, T], F32, tag="psB", name="pe3")
            for i in range(3):
                c = 3 * grp + i
                nc.tensor.matmul(ps3[:, i, :], wpatch_sb[:, 128 * c:128 * (c + 1)],
                                 patches_sb[:], start=(i == 0), stop=(i == 2))
            pe_ps.append(ps3)
        drain([p[:, :, :] for p in pe_ps], None, H, lnin1,
              add_sb=[pos_sb[:, 0:3, :], pos_sb[:, 3:6, :]])

        for l in range(depth):
            # weight loads for this layer (emitted first so DMA starts early)
            wvec_t = wsump.tile([33, 2688], F16, tag="wvec")
            nc.sync.dma_start(out=wvec_t[0:33:32, :], in_=wvecs[l])
            # DMA queue is FIFO; emit in expected buffer-free order so a
            # gate-blocked load never shadows an already-loadable one:
            # spare-slot loads first, then by when the prior layer's tiles
            # die (qkv ~35%, proj ~52%, fc1a ~62%, fc1b ~75%, fc2 late).
            wqkv = [wqkvp.tile([128, 3 * D], F16, tag="wqkv", name="wqkv")
                    for _ in range(NCH)]
            wfc1 = [[wfc1p.tile([128, MLP // 2], F16, tag="wfc1", name="wfc1")
                     for _ in range(NCH)] for _ in range(2)]
            wproj = [wprojp.tile([128, D], F16, tag="wproj", name="wproj")
                     for _ in range(NCH)]
            wfc2_t = [wfc2p.tile([128, 6, D], F16, tag="wfc2", name="wfc2")
                      for _ in range(4)]

            def dma_qkv(k):
                nc.sync.dma_start(out=wqkv[k][:],
                                  in_=wqkvT[l, 128 * k:128 * (k + 1), :])

            def dma_fc1(h, k):
                nc.sync.dma_start(out=wfc1[h][k][:],
                                  in_=wfc1T[l, 128 * k:128 * (k + 1),
                                            1536 * h:1536 * (h + 1)])

            def dma_proj(k):
                nc.sync.dma_start(out=wproj[k][:],
                                  in_=wprojT[l, 128 * k:128 * (k + 1), :])

            def dma_fc2(q):
                nc.sync.dma_start(
                    out=wfc2_t[q][:],
                    in_=wfc2T[l, 768 * q:768 * (q + 1), :].rearrange(
                        "(k p) j -> p k j", p=128))

            for k in range(NCH):
                dma_qkv(k)
            dma_fc1(0, 0)
            dma_fc1(0, 1)
            dma_fc2(0)
            dma_fc2(1)
            for k in range(NCH):
                dma_proj(k)
            for k in range(2, NCH):
                dma_fc1(0, k)
            for k in range(NCH):
                dma_fc1(1, k)
            dma_fc2(2)
            dma_fc2(3)

            # ---- LN1 stats (x and x^2 already staged in lnin1) ----
            (mu16_1, mu32_1), anb16_1 = ln_stats(lnin1)

            # ---- QKV: Q,K feature-major on raw x16 + K=1 correction ----
            qk16 = qkp.tile([128, 2 * NCH, T], F16, tag="qk")
            ab1 = None
            first = True
            for grp in [0, 2, 1, 3]:
                ps3 = psB.tile([128, 3, T], F32, tag="psB", name="qk3")
                for i in range(3):
                    oc = 3 * grp + i
                    for k in range(NCH):
                        nc.tensor.matmul(ps3[:, i, :],
                                         wqkv[k][:, 128 * oc:128 * (oc + 1)],
                                         lnin1[:, 0, k, :],
                                         start=(k == 0), stop=False)
                    nc.tensor.matmul(ps3[:, i, :],
                                     wvec_t[0:1, 128 * oc:128 * (oc + 1)],
                                     mu16_1[:], start=False, stop=True)
                if first:
                    # emitted after the first group's matmuls: the PE sits
                    # behind them in queue order, so the wait on the DVE
                    # stats chain overlaps that GEMM stream
                    ab1 = ln_bcast(anb16_1)
                    first = False
                nc.vector.tensor_mul(qk16[:, 3 * grp:3 * (grp + 1), :],
                                     ps3[:, :, :], bfree(ab1[:, 0, :], 3))

            # per-image rstd as a column (PE transpose) * mask -> V scale
            csc = []
            for b in range(BL):
                vt = psC.tile([KT, 1], F16, tag="psC", name="vt")
                nc.tensor.matmul(vt[:], anb16_1[0:1, 0, KT * b:KT * (b + 1)],
                                 ones16[0:1, 0:1], is_transpose=True)
                cs = tinyp.tile([KT, 1], F32, tag="csc")
                nc.vector.tensor_mul(cs[:], vt[:], m32_sb[:, b:b + 1])
                csc.append(cs)

            # ---- V token-major per image, raw x16 + correction ----
            v16 = []
            for b in range(BL):
                vps0 = psC.tile([KT, 384], F32, tag="psC", name="vps")
                vps1 = psC.tile([KT, 384], F32, tag="psC", name="vps")
                for k in range(NCH):
                    nc.tensor.matmul(vps0[:],
                                     lnin1[:, 0, k, KT * b:KT * (b + 1)],
                                     wqkv[k][:, 2 * D:2 * D + 384],
                                     start=(k == 0), stop=False)
                nc.tensor.matmul(vps0[:], mu16_1[0:1, KT * b:KT * (b + 1)],
                                 wvec_t[0:1, 2 * D:2 * D + 384],
                                 start=False, stop=True)
                for k in range(NCH):
                    nc.tensor.matmul(vps1[:],
                                     lnin1[:, 0, k, KT * b:KT * (b + 1)],
                                     wqkv[k][:, 2 * D + 384:3 * D],
                                     start=(k == 0), stop=False)
                nc.tensor.matmul(vps1[:], mu16_1[0:1, KT * b:KT * (b + 1)],
                                 wvec_t[0:1, 2 * D + 384:3 * D],
                                 start=False, stop=True)
                v = vp.tile([KT, D], F16, tag="v")
                nc.vector.tensor_scalar_mul(v[:, 0:384], vps0[:], csc[b][:])
                nc.vector.tensor_scalar_mul(v[:, 384:768], vps1[:], csc[b][:])
                v16.append(v)

            # ---- attention; images interleaved, heads grouped by parity.
            # PV runs on the raw exp(S) values: the attn mask is folded into
            # the V drain (row scale) and the 1/rowsum normalization into the
            # ot16 assembly multiply, so the softmax scalar chain never
            # blocks the PE stream.
            # fully pipelined per-(image, head-group) units: each unit's
            # exp->rowsum->recip->mask->bcast->assembly chain hides behind
            # the later units' S/PV matmul stream.  No softmax eps: rowsums
            # of exp() are bounded >= e^-O(1) here, and the reference's
            # +1e-9 shifts them by ~1e-11 relative.
            ot16 = otp.tile([128, NCH, T], F16, tag="ot")
            e16s = [ep.tile([KT, 2, 6 * KT], F16, tag="e", name="e16")
                    for _ in range(BL)]
            rrs = [medp.tile([1, 2, 6 * KT], F16, tag="rr", name="rr")
                   for _ in range(BL)]
            rrbs = [bcp.tile([64, 2, 6 * KT], F16, tag="rb", name="rrb")
                    for _ in range(BL)]
            units = [(b, g) for b in range(BL) for g in range(2)]

            def att_s(b, g):
                sps = psC.tile([KT, 6 * KT], F32, tag="psC", name="sps")
                for j in range(6):
                    nc.tensor.matmul(
                        sps[:, KT * j:KT * (j + 1)],
                        qk16[64 * g:64 * (g + 1), 6 + j, KT * b:KT * (b + 1)],
                        qk16[64 * g:64 * (g + 1), j, KT * b:KT * (b + 1)],
                        start=True, stop=True)
                nc.scalar.activation(e16s[b][:, g, :], sps[:],
                                     AF.Exp, scale=SCALE)

            def att_norm(b, g):
                rps = psC.tile([1, 6 * KT], F32, tag="psC", name="rps")
                nc.tensor.matmul(rps[:], m_sb[:, b:b + 1],
                                 e16s[b][:, g, :], start=True, stop=True)
                with nc.allow_low_precision(reason="softmax norm fp16"):
                    nc.vector.reciprocal(rrs[b][0:1, g, :], rps[:])

            def att_out(b, g):
                ops = psC.tile([64, 6 * KT], F32, tag="psC", name="ops")
                for j in range(6):
                    nc.tensor.matmul(
                        ops[:, KT * j:KT * (j + 1)],
                        v16[b][:, 128 * j + 64 * g:128 * j + 64 * g + 64],
                        e16s[b][:, g, KT * j:KT * (j + 1)],
                        start=True, stop=True)
                rbp = pab.tile([64, 6 * KT], F32, tag="pab", name="rbp")
                nc.tensor.matmul(rbp[:], onesr[:], rrs[b][0:1, g, :],
                                 start=True, stop=True)
                nc.scalar.activation(rrbs[b][:, g, :], rbp[:], AF.Copy)
                nc.vector.tensor_mul(
                    ot16[64 * g:64 * (g + 1), :, KT * b:KT * (b + 1)],
                    ops[:].rearrange("p (j t) -> p j t", j=6),
                    rrbs[b][:, g, :].rearrange("p (j t) -> p j t", j=6))

            # 3-stage software pipeline: slot i runs S/exp of unit i, the
            # rowsum/recip of unit i-1, and the PV/normalize of unit i-2,
            # so every unit's scalar chain hides behind later units' PE work
            for i in range(len(units) + 2):
                if i < len(units):
                    att_s(*units[i])
                if i == 1:
                    # hoist the gelu-set load into the attention phase
                    dmy = tinyp.tile([1, 1], F16, tag="tiny16")
                    nc.scalar.activation(dmy[:], ones16[0:1, 0:1], AF.Gelu)
                if 1 <= i < len(units) + 1:
                    att_norm(*units[i - 1])
                if 2 <= i:
                    att_out(*units[i - 2])

            # ---- proj + residual ----
            # proj split by image: img0's half streams on PE while img1's
            # softmax scalar chain is still finishing
            pj = [psB.tile([128, 3, T], F32, tag="psB", name="pj3")
                  for _ in range(2)]
            for b in range(BL):
                cs = slice(KT * b, KT * (b + 1))
                for grp in range(2):
                    for i in range(3):
                        oc = 3 * grp + i
                        for k in range(NCH):
                            nc.tensor.matmul(pj[grp][:, i, cs],
                                             wproj[k][:, 128 * oc:
                                                      128 * (oc + 1)],
                                             ot16[:, k, cs],
                                             start=(k == 0 and b == 0 and i == 0),
                                             stop=(k == NCH - 1 and b == BL - 1
                                                   and i == 2))
            Hn = hp.tile([128, NCH, T], F32, tag="H")
            lnin2 = lnp.tile([128, 2, NCH, T], F16, tag="lnin")
            drain([pj[0][:, :, :], pj[1][:, :, :]], H, Hn, lnin2)
            H = Hn

            # ---- LN2 + MLP ----
            (mu16_2, mu32_2), anb16_2 = ln_stats(lnin2)
            g16 = gp.tile([128, MCH, T], F16, tag="g")
            ab2 = None
            for grp in range(MCH // 3):
                ps3 = psB.tile([128, 3, T], F32, tag="psB")
                for i in range(3):
                    oc = 3 * grp + i
                    for k in range(NCH):
                        h, col = divmod(128 * oc, 1536)
                        nc.tensor.matmul(ps3[:, i, :],
                                         wfc1[h][k][:, col:col + 128],
                                         lnin2[:, 0, k, :],
                                         start=(k == 0), stop=False)
                    row, coff = divmod(3 * D + 128 * oc, 2688)
                    mu_ap = mu16_2[0:1, :] if row == 0 else mu32_2[32:33, :]
                    nc.tensor.matmul(ps3[:, i, :],
                                     wvec_t[32 * row:32 * row + 1,
                                            coff:coff + 128],
                                     mu_ap, start=False, stop=True)
                if grp == 0:
                    ab2 = ln_bcast(anb16_2)
                nc.vector.tensor_mul(ps3[:, :, :], ps3[:, :, :],
                                     bfree(ab2[:, 0, :], 3))
                nc.scalar.activation(g16[:, 3 * grp:3 * (grp + 1), :], ps3[:, :, :],
                                     AF.Gelu)
            # hoist the exp-set load into the MLP phase (covers the next
            # layer's attention exps and this layer's trailing squares --
            # square lives in every set)
            dmy = tinyp.tile([1, 1], F16, tag="tiny16")
            nc.scalar.activation(dmy[:], ones16[0:1, 0:1], AF.Exp)
            # fc2 with k OUTER in halves so weight k-tiles die right after
            # use and next-layer DMA streams during this stage; two psB
            # accumulator tiles (oc 0-2 / 3-5) drain progressively.
            acc = [psB.tile([128, 3, T], F32, tag="psB", name="acc2")
                   for _ in range(2)]
            Hn = hp.tile([128, NCH, T], F32, tag="H")
            lnin_n = lnp.tile([128, 2, NCH, T], F16, tag="lnin")
            KH = MCH // 2
            for half in range(2):
                for oc in range(NCH):
                    for kk in range(KH):
                        k = half * KH + kk
                        nc.tensor.matmul(acc[oc // 3][:, oc % 3, :],
                                         wfc2_t[k // 6][:, k % 6,
                                                        128 * oc:128 * (oc + 1)],
                                         g16[:, k, :],
                                         start=(k == 0 and oc % 3 == 0),
                                         stop=(k == MCH - 1 and oc % 3 == 2))
                    if half == 1 and oc == 2:
                        # bank A closed: start its fp16 cast + ACT square
                        # while bank B still accumulates on PE
                        nc.vector.tensor_add(lnin_n[:, 0, 0:3, :],
                                             H[:, 0:3, :], acc[0][:, :, :])
                        nc.scalar.activation(lnin_n[:, 1, 0:3, :],
                                             lnin_n[:, 0, 0:3, :], AF.Square)
            nc.vector.tensor_add(lnin_n[:, 0, 3:6, :], H[:, 3:6, :],
                                 acc[1][:, :, :])
            nc.vector.tensor_mul(lnin_n[:, 1, 3:6, :], lnin_n[:, 0, 3:6, :],
                                 lnin_n[:, 0, 3:6, :])
            for g in range(2):
                nc.vector.tensor_add(Hn[:, 3 * g:3 * g + 3, :],
                                     H[:, 3 * g:3 * g + 3, :],
                                     acc[g][:, :, :])
            H = Hn
            lnin1 = lnin_n

        # ---- final LN (fp16 out; host upcasts) + store ----
        (mu16_f, mu32_f), anb16_f = ln_stats(lnin1)
        abf = ln_bcast(anb16_f)
        yf = otp.tile([128, NCH, T], F16, tag="ot", name="yf")
        for grp in range(2):
            sl = slice(3 * grp, 3 * (grp + 1))
            nc.vector.scalar_tensor_tensor(yf[:, sl, :], lnin1[:, 0, sl, :], 1.0,
                                           bfree(abf[:, 1, :], 3),
                                           op0=OP.mult, op1=OP.subtract)
            nc.vector.tensor_mul(yf[:, sl, :], yf[:, sl, :],
                                 bfree(abf[:, 0, :], 3))
            nc.sync.dma_start(out=out_d[:, sl, :], in_=yf[:, sl, :])

    nc.compile()
    return nc


def prep_inputs(inputs, depth=DEPTH):
    """Host-side marshalling. Returns per-core in_maps list."""
    g = {k: np.asarray(v) for k, v in inputs.items()}
    x = g["x"].astype(np.float32)
    noise = g["noise"].astype(np.float32)
    attn_mask = g["attn_mask"].astype(np.float32)
    ids_y = g["pos_embed_y_ids"].astype(np.int64)

    ids_shuffle = np.argsort(noise, axis=1, kind="stable")
    ids_keep = ids_shuffle[:, :LEN_KEEP]                      # (B, 75)

    patches = x.reshape(B, GH, GW, Q_).reshape(B, L, Q_)      # (B, 300, 100)
    mask_l = attn_mask.reshape(B, L)

    # pos vector per patch: [pos_y(384) | pos_x(384) * mask]
    pos_y = g["pos_y_table"].astype(np.float32)               # (13, 384)
    pos_x = g["pos_embed_x"].astype(np.float32)[0]            # (26, 384)
    ids_y_l = ids_y.reshape(B, L)
    gw_idx = np.tile(np.arange(GW), GH)                       # (300,)
    pos_full = np.zeros((B, L, D), np.float32)
    pos_full[:, :, :D // 2] = pos_y[ids_y_l]
    pos_full[:, :, D // 2:] = mask_l[:, :, None] * pos_x[gw_idx + 1][None]

    cls_vec = g["cls_token"].astype(np.float32).reshape(D).copy()
    cls_vec[D // 2:] += pos_x[0]

    wqkvT = np.ascontiguousarray(
        g["qkv_w"].astype(np.float32).transpose(0, 2, 1)[:depth]).astype(np.float16)
    wprojT = np.ascontiguousarray(
        g["proj_w"].astype(np.float32).transpose(0, 2, 1)[:depth]).astype(np.float16)
    wfc1T = np.ascontiguousarray(
        g["fc1_w"].astype(np.float32).transpose(0, 2, 1)[:depth]).astype(np.float16)
    wfc2T = np.ascontiguousarray(
        g["fc2_w"].astype(np.float32).transpose(0, 2, 1)[:depth]).astype(np.float16)
    wpatchT = np.ascontiguousarray(
        g["conv_w"].astype(np.float32).reshape(D, Q_).T).astype(np.float16)

    wsqn = -wqkvT.astype(np.float32).sum(axis=1).astype(np.float16)  # (depth, 3D)
    wsf1n = -wfc1T.astype(np.float32).sum(axis=1).astype(np.float16)
    wvecs = np.ascontiguousarray(np.concatenate([wsqn, wsf1n], axis=1)
                                 .reshape(depth, 2, 2688))

    in_maps = []
    for core in range(NCORES):
        patchesT = np.zeros((PIX, T), np.float16)
        posT = np.zeros((D, T), np.float32)
        mv = np.zeros((BL, KT), np.float16)
        for b in range(BL):
            img = core * BL + b
            sel = ids_keep[img]                               # (75,)
            patchesT[:, KT * b + 1:KT * (b + 1)] = patches[img, sel].T
            posT[:, KT * b] = cls_vec
            posT[:, KT * b + 1:KT * (b + 1)] = pos_full[img, sel].T
            mv[b, 0] = 1.0
            mv[b, 1:] = mask_l[img, np.sort(sel)]
        mvf = np.tile(mv.astype(np.float16)[:, None, :], (1, 12, 1)).reshape(
            1, BL, 2, 6 * KT)
        in_maps.append({
            "patchesT": patchesT,
            "posT": posT.reshape(NCH, 128, T),
            "mvec": mv,
            "mvecf": mvf,
            "wpatchT": wpatchT,
            "wqkvT": wqkvT,
            "wprojT": wprojT,
            "wfc1T": wfc1T,
            "wfc2T": wfc2T,
            "wvecs": wvecs,
        })
    return in_maps


_NC_CACHE = {}


def kernel(**inputs):
    if "nc" not in _NC_CACHE:
        _NC_CACHE["nc"] = build()
    nc = _NC_CACHE["nc"]
    in_maps = prep_inputs(inputs)
    res = run_bass_kernel_spmd(nc, in_maps, list(range(NCORES)))
    # device output is feature-major [p, c, t] with feature = 128*c + p
    outs = []
    for i in range(NCORES):
        a = res.results[i]["out"].reshape(128, NCH, T).astype(np.float32)
        a = a.transpose(1, 0, 2).reshape(D, T)
        outs.append(np.ascontiguousarray(a.T).reshape(BL, KT, D))
    return np.concatenate(outs, axis=0).astype(np.float32)


# revision 59
# speedup vs baseline: 1.0120x; 1.0013x over previous
"""MAE ViT encoder (nn_MaskedAutoencoderViT) Trainium2 Bass kernel.

Strategy: data-parallel over batch (16 images -> 8 cores x 2 images).
Feature-major activation layout on chip: activations stored transposed as
[128 partitions (d chunk), 6 chunks, 152 tokens] so every matmul is
weight-stationary (lhsT = 128x128 weight tile, rhs = activation columns)
with zero on-device transposes.  Attention is computed in transposed form
(S^T = (K^T)-stationary @ Q^T), softmax uses the structure
exp(att)/ (sum + 1e-9) (the reference's global-max subtraction cancels in
the normalization up to ~1e-10 relative, far below fp32 noise).
Matmul operands in fp16 (full PE rate, 11-bit mantissa), accumulation and
residual stream in fp32.

Scheduling structure (all serialization chains measured in the timeline
cost model):
- weight streaming: one DMA per k-chunk / fc2-quarter; DMA issue and the
  shared HWDGE device serialize per-instruction, so chunk-merged DMAs
  keep both off the critical path (transfer cost is bytes-based).
- each residual drain runs twice in parallel: Pool produces the fp32
  stream (H, double-buffered), DVE produces the fp16 matmul operand x16
  directly into the LN staging tile; ACT squares follow per chunk, so
  LN stats and the next GEMM both unblock ~0.6us after the psum stop.
- LN mean/rstd travel to all partitions via a 1x128 ones-matmul into
  PSUM (~130ns on the idle-ish PE) instead of gpsimd partition_broadcast
  (~1.4us on Pool).
- QKV and fc1 run on raw x16 with a K=1 colsum*mu correction row; V runs
  on raw x16 too, with the correction applied token-major and rstd folded
  into the per-token mask scale (needs one 1-col PE transpose).
- exp/gelu activation-table loads are hoisted off the critical path by
  dummy [1,1] activations issued while ACT is idle.
Host side does only data marshalling: noise argsort, patch gather,
pos-embed gathers, weight transposition + fp16 cast.
"""
import numpy as np
from contextlib import ExitStack

import concourse.bass as bass
import concourse.bacc as bacc
import concourse.mybir as mybir
import concourse.tile as tile
import bass_rust as _bass_rust
from concourse.bass_utils import run_bass_kernel_spmd
from concourse.hw_specs import get_activation_tables


class _Bacc(bacc.Bacc):
    """Bacc whose ACT-table-load pass prefers multi-function sets.

    The stock pass picks the first table set containing each activation
    function, which sends Ln to `natural_log` and Exp to `exp_and_others`
    and thrashes the table RAM inside every layernorm.  Reordering the
    set dict so `natural_log_exp_and_others` comes first makes Ln and Exp
    share one resident set (2 loads per layer total: exp-set <-> gelu-set).
    """

    def insert_act_table_loads(self):
        has_activation = any(
            isinstance(i, mybir.InstActivation)
            for b in self.main_func.blocks
            for i in b.instructions
        )
        if not has_activation:
            return
        # Keep the canonical set order (set ids are positional and the
        # runtime resolves them canonically) but hide Exp/Ln from every
        # other set so the chooser lands on the combined one.
        tabs = get_activation_tables(self.m.arch)
        items = []
        for k, v in tabs.items():
            if k != "natural_log_exp_and_others":
                v = {f for f in v if f.name not in ("Exp", "Ln")}
            items.append((k, v))
        _bass_rust.insert_act_table_loads(self, items)

F16 = mybir.dt.float16
F32 = mybir.dt.float32
AF = mybir.ActivationFunctionType
OP = mybir.AluOpType

# --- model config (hardcoded from the problem spec) ---
B, C_IN, H_IN, W_IN = 16, 1, 12, 2500
P_, Q_ = 1, 100
D, NH, DEPTH = 768, 12, 12
GH, GW = 12, 25
L = GH * GW                      # 300
LEN_KEEP = 75
HD = D // NH                     # 64
SCALE = HD ** -0.5               # 0.125
EPS_LN = 1e-5
MLP = 4 * D                      # 3072

NCORES = 8
BL = B // NCORES                 # 2 images per core
KT = 1 + LEN_KEEP                # 76 tokens per image
T = BL * KT                      # 152 token columns per core
NCH = D // 128                   # 6 feature chunks
MCH = MLP // 128                 # 24 mlp chunks
PIX = P_ * Q_                    # 100 pixels per patch


def bfree(ap, n, at=1):
    """Insert a 0-step (broadcast) free dim of size n at position `at`."""
    new_ap = list(ap.ap[:at]) + [[0, n]] + list(ap.ap[at:])
    return bass.AP(tensor=ap.tensor, offset=ap.offset, ap=new_ap)


def build(depth=DEPTH):
    nc = _Bacc("TRN2", target_bir_lowering=False, debug=False,
               num_devices=NCORES)

    # DRAM I/O
    # patches and the patch-embed weight share one input DMA
    pwT = nc.dram_tensor("pwT", [PIX, T + D], F16, kind="ExternalInput").ap()
    posT = nc.dram_tensor("posT", [NCH, 128, T], F32, kind="ExternalInput").ap()
    mvec = nc.dram_tensor("mvec", [BL, KT], F16, kind="ExternalInput").ap()
    mvecf = nc.dram_tensor("mvecf", [1, BL, 2, 6 * KT], F16, kind="ExternalInput").ap()
    wqkvT = nc.dram_tensor("wqkvT", [depth, D, 3 * D], F16,
                           kind="ExternalInput").ap()
    wprojT = nc.dram_tensor("wprojT", [depth, D, D], F16,
                            kind="ExternalInput").ap()
    wfc1T = nc.dram_tensor("wfc1T", [depth, D, MLP], F16, kind="ExternalInput").ap()
    wfc2T = nc.dram_tensor("wfc2T", [depth, MLP, D], F16, kind="ExternalInput").ap()
    # [0:2D]: -colsum(Wq|Wk); [2D:3D]: -colsum(Wv); [3D:]: -colsum(Wfc1),
    # packed two partition rows of 2688 so the tile costs half the columns
    wvecs = nc.dram_tensor("wvecs", [depth, 2, 2688], F16,
                           kind="ExternalInput").ap()
    out_d = nc.dram_tensor("out", [128, NCH, T], F16, kind="ExternalOutput").ap()

    with tile.TileContext(nc) as tc, ExitStack() as ctx:
        pool = lambda name, bufs, **kw: ctx.enter_context(
            tc.tile_pool(name=name, bufs=bufs, **kw))

        const = pool("const", 1)
        hp = pool("hp", 2)
        lnp = pool("lnp", 2)
        qkp = pool("qkp", 1)
        vp = pool("vp", 2)
        ep = pool("ep", 2)
        otp = pool("otp", 1)
        gp = pool("gp", 1)
        tinyp = pool("tinyp", 5)
        medp = pool("medp", 3)
        bcp = pool("bcp", 2)
        wsump = pool("wsump", 2)
        wqkvp = pool("wqkvp", 7)
        wprojp = pool("wprojp", 7)
        wfc1p = pool("wfc1p", 14)
        wfc2p = pool("wfc2p", 6)

        psB = pool("psB", 3, space="PSUM")
        psC = pool("psC", 4, space="PSUM")
        pab = pool("pab", 1, space="PSUM")

        # constants
        ones16 = const.tile([128, 1], F16)
        nc.vector.memset(ones16[:], 1.0)
        onesr = const.tile([1, 64], F16)
        nc.vector.memset(onesr[:], 1.0)
        onesr128 = const.tile([1, 128], F16)
        nc.vector.memset(onesr128[:], 1.0)
        eps_t = const.tile([1, 1], F32)
        nc.vector.memset(eps_t[:], EPS_LN)
        # pre-load the ln/exp table set while the input DMAs stream; the
        # patch-embed squares then run from this set with no reload
        dmy0 = tinyp.tile([1, 1], F16, tag="tiny16")
        nc.scalar.activation(dmy0[:], eps_t[:], AF.Ln)

        # static inputs; the patch-embed-only tiles borrow weight-pool
        # slots (they die right after the patch embed, so the slots return
        # to the steady-state weight stream)
        pw_sb = wsump.tile([PIX, T + D], F16, tag="wvec", name="pw_sb")
        nc.sync.dma_start(out=pw_sb[:], in_=pwT[:])
        pos_sb = wfc2p.tile([128, NCH, T], F32, tag="wfc2", name="pos_sb")
        nc.sync.dma_start(out=pos_sb[:], in_=posT.rearrange("c p t -> p c t"))
        m_sb = const.tile([KT, BL], F16)
        nc.sync.dma_start(out=m_sb[:], in_=mvec.rearrange("b t -> t b"))
        m32_sb = const.tile([KT, BL], F32)
        nc.vector.tensor_copy(m32_sb[:], m_sb[:])
        mf_sb = const.tile([1, BL, 2, 6 * KT], F16)
        nc.sync.dma_start(out=mf_sb[:], in_=mvecf[:])

        def drain(ps_aps, h_old, h_new, lnin_new, add_sb=None):
            """Residual drain, both 3-chunk groups: DVE writes the fp16
            matmul copies into lnin_new[:,0] first (the critical
            successors), then squares the second group at fp16 2x rate
            while ACT squares the first; the fp32 stream updates into
            h_new go last (they are not needed until the next drain).
            Pool has no PSUM port on TRN2, so everything lives on DVE/ACT.
            add_sb: use these sbuf tensors instead of h_old (patch embed)."""
            sls = [slice(0, 3), slice(3, 6)]
            srcs = [add_sb[g] if add_sb is not None else h_old[:, sls[g], :]
                    for g in range(2)]
            for g in range(2):
                nc.vector.tensor_add(lnin_new[:, 0, sls[g], :], srcs[g],
                                     ps_aps[g])
                if g == 0:
                    nc.scalar.activation(lnin_new[:, 1, sls[0], :],
                                         lnin_new[:, 0, sls[0], :], AF.Square)
            nc.vector.tensor_mul(lnin_new[:, 1, sls[1], :],
                                 lnin_new[:, 0, sls[1], :],
                                 lnin_new[:, 0, sls[1], :])
            for g in range(2):
                nc.vector.tensor_add(h_new[:, sls[g], :], srcs[g], ps_aps[g])

        def ln_stats(lnin):
            """lnin [128,2,NCH,T] fp16 prefilled with [x | x^2].
            Returns (mu16 [1,T] f16, ab psum [128,2,T] f32 with
            [:,0]=rstd, [:,1]=mu broadcast, anb16 [1,2,T] f16)."""
            st = psC.tile([1, 2, T], F32, tag="psC", name="st")
            for c in range(NCH):
                nc.tensor.matmul(st[:], ones16[:, 0:1], lnin[:, :, c, :],
                                 start=(c == 0), stop=(c == NCH - 1))
            mean = tinyp.tile([1, T], F32, tag="tiny")
            nc.vector.tensor_scalar_mul(mean[:], st[0:1, 0, :], 1.0 / D)
            msq = tinyp.tile([1, T], F32, tag="tiny")
            nc.vector.tensor_mul(msq[:], mean[:], mean[:])
            v = tinyp.tile([1, T], F32, tag="tiny")
            nc.vector.scalar_tensor_tensor(v[:], st[0:1, 1, :], 1.0 / D, msq[:],
                                           op0=OP.mult, op1=OP.subtract)
            # rstd = exp(-0.5*ln(v+eps)) on ACT: Ln and Exp live in the
            # same table set (see _Bacc), and this replaces the 8-op DVE
            # Newton chain (each [1,T] op pays ~150ns of sem latency)
            anb16 = medp.tile([1, 2, T], F16, tag="anb")
            lnv = tinyp.tile([1, T], F32, tag="tiny")
            nc.scalar.activation(lnv[:], v[:], AF.Ln, bias=eps_t[0:1, 0:1])
            nc.scalar.activation(anb16[0:1, 0, :], lnv[:], AF.Exp, scale=-0.5)
            nc.vector.tensor_copy(anb16[0:1, 1, :], mean[:])
            # mu16 on two partition rows (the packed wvec correction rows
            # need a matching rhs base partition)
            # fast path row-0 copy feeds the early corrections; the
            # row-32 replica (packed-wvec fc1 slices) comes from a Pool
            # broadcast that never gates the LN chain
            mu16 = tinyp.tile([1, T], F16, tag="tiny16")
            nc.vector.tensor_copy(mu16[:], mean[:])
            mu32 = tinyp.tile([33, T], F16, tag="mu2")
            nc.gpsimd.partition_broadcast(mu32[:], mu16[:])
            return (mu16, mu32), anb16

        def ln_bcast(anb16):
            """Broadcast [1,2,T] across partitions via PE ones-matmul,
            then ACT-copy PSUM->SBUF (the drains' other operand is PSUM,
            and only one PSUM operand per DVE op is legal)."""
            ab = pab.tile([128, 2, T], F32, tag="pab")
            nc.tensor.matmul(ab[:], onesr128[:], anb16[:],
                             start=True, stop=True)
            ab_sb = bcp.tile([128, 2, T], F16, tag="bc")
            nc.scalar.activation(ab_sb[:], ab[:], AF.Copy)
            return ab_sb

        # residual stream, feature-major fp32 (double-buffered so the
        # Pool-side update overlaps the DVE-side fp16 cast)
        H = hp.tile([128, NCH, T], F32, tag="H")
        lnin1 = lnp.tile([128, 2, NCH, T], F16, tag="lnin")

        # ---- patch embed + pos add ----
        pe_ps = []
        for grp in range(2):
            ps3 = psB.tile([128, 3, T], F32, tag="psB", name="pe3")
            for i in range(3):
                c = 3 * grp + i
                nc.tensor.matmul(ps3[:, i, :],
                                 pw_sb[:, T + 128 * c:T + 128 * (c + 1)],
                                 pw_sb[:, 0:T], start=(i == 0), stop=(i == 2))
            pe_ps.append(ps3)
        drain([p[:, :, :] for p in pe_ps], None, H, lnin1,
              add_sb=[pos_sb[:, 0:3, :], pos_sb[:, 3:6, :]])

        for l in range(depth):
            # weight loads for this layer (emitted first so DMA starts early)
            wvec_t = wsump.tile([33, 2688], F16, tag="wvec")
            nc.sync.dma_start(out=wvec_t[0:33:32, :], in_=wvecs[l])
            # DMA queue is FIFO; emit in expected buffer-free order so a
            # gate-blocked load never shadows an already-loadable one:
            # spare-slot loads first, then by when the prior layer's tiles
            # die (qkv ~35%, proj ~52%, fc1a ~62%, fc1b ~75%, fc2 late).
            wqkv = [wqkvp.tile([128, 3 * D], F16, tag="wqkv", name="wqkv")
                    for _ in range(NCH)]
            wfc1 = [[wfc1p.tile([128, MLP // 2], F16, tag="wfc1", name="wfc1")
                     for _ in range(NCH)] for _ in range(2)]
            wproj = [wprojp.tile([128, D], F16, tag="wproj", name="wproj")
                     for _ in range(NCH)]
            wfc2_t = [wfc2p.tile([128, 6, D], F16, tag="wfc2", name="wfc2")
                      for _ in range(4)]

            def dma_qkv(k):
                nc.sync.dma_start(out=wqkv[k][:],
                                  in_=wqkvT[l, 128 * k:128 * (k + 1), :])

            def dma_fc1(h, k):
                nc.sync.dma_start(out=wfc1[h][k][:],
                                  in_=wfc1T[l, 128 * k:128 * (k + 1),
                                            1536 * h:1536 * (h + 1)])

            def dma_proj(k):
                nc.sync.dma_start(out=wproj[k][:],
                                  in_=wprojT[l, 128 * k:128 * (k + 1), :])

            def dma_fc2(q):
                nc.sync.dma_start(
                    out=wfc2_t[q][:],
                    in_=wfc2T[l, 768 * q:768 * (q + 1), :].rearrange(
                        "(k p) j -> p k j", p=128))

            for k in range(NCH):
                dma_qkv(k)
            dma_fc1(0, 0)
            dma_fc1(0, 1)
            dma_fc2(0)
            dma_fc2(1)
            for k in range(NCH):
                dma_proj(k)
            for k in range(2, NCH):
                dma_fc1(0, k)
            for k in range(NCH):
                dma_fc1(1, k)
            dma_fc2(2)
            dma_fc2(3)

            # ---- LN1 stats (x and x^2 already staged in lnin1) ----
            (mu16_1, mu32_1), anb16_1 = ln_stats(lnin1)

            # ---- QKV: Q,K feature-major on raw x16 + K=1 correction ----
            qk16 = qkp.tile([128, 2 * NCH, T], F16, tag="qk")
            ab1 = None
            first = True
            for grp in [0, 2, 1, 3]:
                ps3 = psB.tile([128, 3, T], F32, tag="psB", name="qk3")
                for i in range(3):
                    oc = 3 * grp + i
                    for k in range(NCH):
                        nc.tensor.matmul(ps3[:, i, :],
                                         wqkv[k][:, 128 * oc:128 * (oc + 1)],
                                         lnin1[:, 0, k, :],
                                         start=(k == 0), stop=False)
                    nc.tensor.matmul(ps3[:, i, :],
                                     wvec_t[0:1, 128 * oc:128 * (oc + 1)],
                                     mu16_1[:], start=False, stop=True)
                if first:
                    # emitted after the first group's matmuls: the PE sits
                    # behind them in queue order, so the wait on the DVE
                    # stats chain overlaps that GEMM stream
                    ab1 = ln_bcast(anb16_1)
                    first = False
                nc.vector.tensor_mul(qk16[:, 3 * grp:3 * (grp + 1), :],
                                     ps3[:, :, :], bfree(ab1[:, 0, :], 3))

            # per-image rstd as a column (PE transpose) * mask -> V scale
            csc = []
            for b in range(BL):
                vt = psC.tile([KT, 1], F16, tag="psC", name="vt")
                nc.tensor.matmul(vt[:], anb16_1[0:1, 0, KT * b:KT * (b + 1)],
                                 ones16[0:1, 0:1], is_transpose=True)
                cs = tinyp.tile([KT, 1], F32, tag="csc")
                nc.vector.tensor_mul(cs[:], vt[:], m32_sb[:, b:b + 1])
                csc.append(cs)

            # ---- V token-major per image, raw x16 + correction ----
            v16 = []
            for b in range(BL):
                vps0 = psC.tile([KT, 384], F32, tag="psC", name="vps")
                vps1 = psC.tile([KT, 384], F32, tag="psC", name="vps")
                for k in range(NCH):
                    nc.tensor.matmul(vps0[:],
                                     lnin1[:, 0, k, KT * b:KT * (b + 1)],
                                     wqkv[k][:, 2 * D:2 * D + 384],
                                     start=(k == 0), stop=False)
                nc.tensor.matmul(vps0[:], mu16_1[0:1, KT * b:KT * (b + 1)],
                                 wvec_t[0:1, 2 * D:2 * D + 384],
                                 start=False, stop=True)
                for k in range(NCH):
                    nc.tensor.matmul(vps1[:],
                                     lnin1[:, 0, k, KT * b:KT * (b + 1)],
                                     wqkv[k][:, 2 * D + 384:3 * D],
                                     start=(k == 0), stop=False)
                nc.tensor.matmul(vps1[:], mu16_1[0:1, KT * b:KT * (b + 1)],
                                 wvec_t[0:1, 2 * D + 384:3 * D],
                                 start=False, stop=True)
                v = vp.tile([KT, D], F16, tag="v")
                nc.vector.tensor_scalar_mul(v[:, 0:384], vps0[:], csc[b][:])
                nc.vector.tensor_scalar_mul(v[:, 384:768], vps1[:], csc[b][:])
                v16.append(v)

            # ---- attention; images interleaved, heads grouped by parity.
            # PV runs on the raw exp(S) values: the attn mask is folded into
            # the V drain (row scale) and the 1/rowsum normalization into the
            # ot16 assembly multiply, so the softmax scalar chain never
            # blocks the PE stream.
            # fully pipelined per-(image, head-group) units: each unit's
            # exp->rowsum->recip->mask->bcast->assembly chain hides behind
            # the later units' S/PV matmul stream.  No softmax eps: rowsums
            # of exp() are bounded >= e^-O(1) here, and the reference's
            # +1e-9 shifts them by ~1e-11 relative.
            ot16 = otp.tile([128, NCH, T], F16, tag="ot")
            e16s = [ep.tile([KT, 2, 6 * KT], F16, tag="e", name="e16")
                    for _ in range(BL)]
            rrs = [medp.tile([1, 2, 6 * KT], F16, tag="rr", name="rr")
                   for _ in range(BL)]
            rrbs = [bcp.tile([64, 2, 6 * KT], F16, tag="rb", name="rrb")
                    for _ in range(BL)]
            units = [(b, g) for b in range(BL) for g in range(2)]

            def att_s(b, g):
                sps = psC.tile([KT, 6 * KT], F32, tag="psC", name="sps")
                for j in range(6):
                    nc.tensor.matmul(
                        sps[:, KT * j:KT * (j + 1)],
                        qk16[64 * g:64 * (g + 1), 6 + j, KT * b:KT * (b + 1)],
                        qk16[64 * g:64 * (g + 1), j, KT * b:KT * (b + 1)],
                        start=True, stop=True)
                nc.scalar.activation(e16s[b][:, g, :], sps[:],
                                     AF.Exp, scale=SCALE)

            def att_norm(b, g):
                rps = psC.tile([1, 6 * KT], F32, tag="psC", name="rps")
                nc.tensor.matmul(rps[:], m_sb[:, b:b + 1],
                                 e16s[b][:, g, :], start=True, stop=True)
                with nc.allow_low_precision(reason="softmax norm fp16"):
                    nc.vector.reciprocal(rrs[b][0:1, g, :], rps[:])

            def att_out(b, g):
                ops = psC.tile([64, 6 * KT], F32, tag="psC", name="ops")
                for j in range(6):
                    nc.tensor.matmul(
                        ops[:, KT * j:KT * (j + 1)],
                        v16[b][:, 128 * j + 64 * g:128 * j + 64 * g + 64],
                        e16s[b][:, g, KT * j:KT * (j + 1)],
                        start=True, stop=True)
                rbp = pab.tile([64, 6 * KT], F32, tag="pab", name="rbp")
                nc.tensor.matmul(rbp[:], onesr[:], rrs[b][0:1, g, :],
                                 start=True, stop=True)
                nc.scalar.activation(rrbs[b][:, g, :], rbp[:], AF.Copy)
                nc.vector.tensor_mul(
                    ot16[64 * g:64 * (g + 1), :, KT * b:KT * (b + 1)],
                    ops[:].rearrange("p (j t) -> p j t", j=6),
                    rrbs[b][:, g, :].rearrange("p (j t) -> p j t", j=6))

            # 3-stage software pipeline: slot i runs S/exp of unit i, the
            # rowsum/recip of unit i-1, and the PV/normalize of unit i-2,
            # so every unit's scalar chain hides behind later units' PE work
            for i in range(len(units) + 2):
                if i < len(units):
                    att_s(*units[i])
                if i == 1:
                    # hoist the gelu-set load into the attention phase
                    dmy = tinyp.tile([1, 1], F16, tag="tiny16")
                    nc.scalar.activation(dmy[:], ones16[0:1, 0:1], AF.Gelu)
                if 1 <= i < len(units) + 1:
                    att_norm(*units[i - 1])
                if 2 <= i:
                    att_out(*units[i - 2])

            # ---- proj + residual ----
            # proj split by image: img0's half streams on PE while img1's
            # softmax scalar chain is still finishing
            pj = [psB.tile([128, 3, T], F32, tag="psB", name="pj3")
                  for _ in range(2)]
            for b in range(BL):
                cs = slice(KT * b, KT * (b + 1))
                for grp in range(2):
                    for i in range(3):
                        oc = 3 * grp + i
                        for k in range(NCH):
                            nc.tensor.matmul(pj[grp][:, i, cs],
                                             wproj[k][:, 128 * oc:
                                                      128 * (oc + 1)],
                                             ot16[:, k, cs],
                                             start=(k == 0 and b == 0 and i == 0),
                                             stop=(k == NCH - 1 and b == BL - 1
                                                   and i == 2))
            Hn = hp.tile([128, NCH, T], F32, tag="H")
            lnin2 = lnp.tile([128, 2, NCH, T], F16, tag="lnin")
            drain([pj[0][:, :, :], pj[1][:, :, :]], H, Hn, lnin2)
            H = Hn

            # ---- LN2 + MLP ----
            (mu16_2, mu32_2), anb16_2 = ln_stats(lnin2)
            g16 = gp.tile([128, MCH, T], F16, tag="g")
            ab2 = None
            for grp in range(MCH // 3):
                ps3 = psB.tile([128, 3, T], F32, tag="psB")
                for i in range(3):
                    oc = 3 * grp + i
                    for k in range(NCH):
                        h, col = divmod(128 * oc, 1536)
                        nc.tensor.matmul(ps3[:, i, :],
                                         wfc1[h][k][:, col:col + 128],
                                         lnin2[:, 0, k, :],
                                         start=(k == 0), stop=False)
                    row, coff = divmod(3 * D + 128 * oc, 2688)
                    mu_ap = mu16_2[0:1, :] if row == 0 else mu32_2[32:33, :]
                    nc.tensor.matmul(ps3[:, i, :],
                                     wvec_t[32 * row:32 * row + 1,
                                            coff:coff + 128],
                                     mu_ap, start=False, stop=True)
                if grp == 0:
                    ab2 = ln_bcast(anb16_2)
                nc.vector.tensor_mul(ps3[:, :, :], ps3[:, :, :],
                                     bfree(ab2[:, 0, :], 3))
                nc.scalar.activation(g16[:, 3 * grp:3 * (grp + 1), :], ps3[:, :, :],
                                     AF.Gelu)
            # hoist the exp-set load into the MLP phase (covers the next
            # layer's attention exps and this layer's trailing squares --
            # square lives in every set)
            dmy = tinyp.tile([1, 1], F16, tag="tiny16")
            nc.scalar.activation(dmy[:], ones16[0:1, 0:1], AF.Exp)
            # fc2 with k OUTER in halves so weight k-tiles die right after
            # use and next-layer DMA streams during this stage; two psB
            # accumulator tiles (oc 0-2 / 3-5) drain progressively.
            acc = [psB.tile([128, 3, T], F32, tag="psB", name="acc2")
                   for _ in range(2)]
            Hn = hp.tile([128, NCH, T], F32, tag="H")
            lnin_n = lnp.tile([128, 2, NCH, T], F16, tag="lnin")
            KH = MCH // 2
            for half in range(2):
                for oc in range(NCH):
                    for kk in range(KH):
                        k = half * KH + kk
                        nc.tensor.matmul(acc[oc // 3][:, oc % 3, :],
                                         wfc2_t[k // 6][:, k % 6,
                                                        128 * oc:128 * (oc + 1)],
                                         g16[:, k, :],
                                         start=(k == 0 and oc % 3 == 0),
                                         stop=(k == MCH - 1 and oc % 3 == 2))
                    if half == 1 and oc == 2:
                        # bank A closed: start its fp16 cast + ACT square
                        # while bank B still accumulates on PE
                        nc.vector.tensor_add(lnin_n[:, 0, 0:3, :],
                                             H[:, 0:3, :], acc[0][:, :, :])
                        nc.scalar.activation(lnin_n[:, 1, 0:3, :],
                                             lnin_n[:, 0, 0:3, :], AF.Square)
            nc.vector.tensor_add(lnin_n[:, 0, 3:6, :], H[:, 3:6, :],
                                 acc[1][:, :, :])
            nc.vector.tensor_mul(lnin_n[:, 1, 3:6, :], lnin_n[:, 0, 3:6, :],
                                 lnin_n[:, 0, 3:6, :])
            for g in range(2):
                nc.vector.tensor_add(Hn[:, 3 * g:3 * g + 3, :],
                                     H[:, 3 * g:3 * g + 3, :],
                                     acc[g][:, :, :])
            H = Hn
            lnin1 = lnin_n

        # ---- final LN (fp16 out; host upcasts) + store ----
        (mu16_f, mu32_f), anb16_f = ln_stats(lnin1)
        abf = ln_bcast(anb16_f)
        yf = otp.tile([128, NCH, T], F16, tag="ot", name="yf")
        for grp in range(2):
            sl = slice(3 * grp, 3 * (grp + 1))
            nc.vector.scalar_tensor_tensor(yf[:, sl, :], lnin1[:, 0, sl, :], 1.0,
                                           bfree(abf[:, 1, :], 3),
                                           op0=OP.mult, op1=OP.subtract)
            nc.vector.tensor_mul(yf[:, sl, :], yf[:, sl, :],
                                 bfree(abf[:, 0, :], 3))
            nc.sync.dma_start(out=out_d[:, sl, :], in_=yf[:, sl, :])

    nc.compile()
    return nc


def prep_inputs(inputs, depth=DEPTH):
    """Host-side marshalling. Returns per-core in_maps list."""
    g = {k: np.asarray(v) for k, v in inputs.items()}
    x = g["x"].astype(np.float32)
    noise = g["noise"].astype(np.float32)
    attn_mask = g["attn_mask"].astype(np.float32)
    ids_y = g["pos_embed_y_ids"].astype(np.int64)

    ids_shuffle = np.argsort(noise, axis=1, kind="stable")
    ids_keep = ids_shuffle[:, :LEN_KEEP]                      # (B, 75)

    patches = x.reshape(B, GH, GW, Q_).reshape(B, L, Q_)      # (B, 300, 100)
    mask_l = attn_mask.reshape(B, L)

    # pos vector per patch: [pos_y(384) | pos_x(384) * mask]
    pos_y = g["pos_y_table"].astype(np.float32)               # (13, 384)
    pos_x = g["pos_embed_x"].astype(np.float32)[0]            # (26, 384)
    ids_y_l = ids_y.reshape(B, L)
    gw_idx = np.tile(np.arange(GW), GH)                       # (300,)
    pos_full = np.zeros((B, L, D), np.float32)
    pos_full[:, :, :D // 2] = pos_y[ids_y_l]
    pos_full[:, :, D // 2:] = mask_l[:, :, None] * pos_x[gw_idx + 1][None]

    cls_vec = g["cls_token"].astype(np.float32).reshape(D).copy()
    cls_vec[D // 2:] += pos_x[0]

    wqkvT = np.ascontiguousarray(
        g["qkv_w"].astype(np.float32).transpose(0, 2, 1)[:depth]).astype(np.float16)
    wprojT = np.ascontiguousarray(
        g["proj_w"].astype(np.float32).transpose(0, 2, 1)[:depth]).astype(np.float16)
    wfc1T = np.ascontiguousarray(
        g["fc1_w"].astype(np.float32).transpose(0, 2, 1)[:depth]).astype(np.float16)
    wfc2T = np.ascontiguousarray(
        g["fc2_w"].astype(np.float32).transpose(0, 2, 1)[:depth]).astype(np.float16)
    wpatchT = np.ascontiguousarray(
        g["conv_w"].astype(np.float32).reshape(D, Q_).T).astype(np.float16)

    wsqn = -wqkvT.astype(np.float32).sum(axis=1).astype(np.float16)  # (depth, 3D)
    wsf1n = -wfc1T.astype(np.float32).sum(axis=1).astype(np.float16)
    wvecs = np.ascontiguousarray(np.concatenate([wsqn, wsf1n], axis=1)
                                 .reshape(depth, 2, 2688))

    in_maps = []
    for core in range(NCORES):
        patchesT = np.zeros((PIX, T), np.float16)
        posT = np.zeros((D, T), np.float32)
        mv = np.zeros((BL, KT), np.float16)
        for b in range(BL):
            img = core * BL + b
            sel = ids_keep[img]                               # (75,)
            patchesT[:, KT * b + 1:KT * (b + 1)] = patches[img, sel].T
            posT[:, KT * b] = cls_vec
            posT[:, KT * b + 1:KT * (b + 1)] = pos_full[img, sel].T
            mv[b, 0] = 1.0
            mv[b, 1:] = mask_l[img, np.sort(sel)]
        mvf = np.tile(mv.astype(np.float16)[:, None, :], (1, 12, 1)).reshape(
            1, BL, 2, 6 * KT)
        in_maps.append({
            "pwT": np.concatenate([patchesT, wpatchT], axis=1),
            "posT": posT.reshape(NCH, 128, T),
            "mvec": mv,
            "mvecf": mvf,
            "wqkvT": wqkvT,
            "wprojT": wprojT,
            "wfc1T": wfc1T,
            "wfc2T": wfc2T,
            "wvecs": wvecs,
        })
    return in_maps


_NC_CACHE = {}


def kernel(**inputs):
    if "nc" not in _NC_CACHE:
        _NC_CACHE["nc"] = build()
    nc = _NC_CACHE["nc"]
    in_maps = prep_inputs(inputs)
    res = run_bass_kernel_spmd(nc, in_maps, list(range(NCORES)))
    # device output is feature-major [p, c, t] with feature = 128*c + p
    outs = []
    for i in range(NCORES):
        a = res.results[i]["out"].reshape(128, NCH, T).astype(np.float32)
        a = a.transpose(1, 0, 2).reshape(D, T)
        outs.append(np.ascontiguousarray(a.T).reshape(BL, KT, D))
    return np.concatenate(outs, axis=0).astype(np.float32)


# revision 62
# speedup vs baseline: 1.0133x; 1.0013x over previous
"""MAE ViT encoder (nn_MaskedAutoencoderViT) Trainium2 Bass kernel.

Strategy: data-parallel over batch (16 images -> 8 cores x 2 images).
Feature-major activation layout on chip: activations stored transposed as
[128 partitions (d chunk), 6 chunks, 152 tokens] so every matmul is
weight-stationary (lhsT = 128x128 weight tile, rhs = activation columns)
with zero on-device transposes.  Attention is computed in transposed form
(S^T = (K^T)-stationary @ Q^T), softmax uses the structure
exp(att)/ (sum + 1e-9) (the reference's global-max subtraction cancels in
the normalization up to ~1e-10 relative, far below fp32 noise).
Matmul operands in fp16 (full PE rate, 11-bit mantissa), accumulation and
residual stream in fp32.

Scheduling structure (all serialization chains measured in the timeline
cost model):
- weight streaming: one DMA per k-chunk / fc2-quarter; DMA issue and the
  shared HWDGE device serialize per-instruction, so chunk-merged DMAs
  keep both off the critical path (transfer cost is bytes-based).
- each residual drain runs twice in parallel: Pool produces the fp32
  stream (H, double-buffered), DVE produces the fp16 matmul operand x16
  directly into the LN staging tile; ACT squares follow per chunk, so
  LN stats and the next GEMM both unblock ~0.6us after the psum stop.
- LN mean/rstd travel to all partitions via a 1x128 ones-matmul into
  PSUM (~130ns on the idle-ish PE) instead of gpsimd partition_broadcast
  (~1.4us on Pool).
- QKV and fc1 run on raw x16 with a K=1 colsum*mu correction row; V runs
  on raw x16 too, with the correction applied token-major and rstd folded
  into the per-token mask scale (needs one 1-col PE transpose).
- exp/gelu activation-table loads are hoisted off the critical path by
  dummy [1,1] activations issued while ACT is idle.
Host side does only data marshalling: noise argsort, patch gather,
pos-embed gathers, weight transposition + fp16 cast.
"""
import numpy as np
from contextlib import ExitStack

import concourse.bass as bass
import concourse.bacc as bacc
import concourse.mybir as mybir
import concourse.tile as tile
import bass_rust as _bass_rust
from concourse.bass_utils import run_bass_kernel_spmd
from concourse.hw_specs import get_activation_tables


class _Bacc(bacc.Bacc):
    """Bacc whose ACT-table-load pass prefers multi-function sets.

    The stock pass picks the first table set containing each activation
    function, which sends Ln to `natural_log` and Exp to `exp_and_others`
    and thrashes the table RAM inside every layernorm.  Reordering the
    set dict so `natural_log_exp_and_others` comes first makes Ln and Exp
    share one resident set (2 loads per layer total: exp-set <-> gelu-set).
    """

    def insert_act_table_loads(self):
        has_activation = any(
            isinstance(i, mybir.InstActivation)
            for b in self.main_func.blocks
            for i in b.instructions
        )
        if not has_activation:
            return
        # Keep the canonical set order (set ids are positional and the
        # runtime resolves them canonically) but hide Exp/Ln from every
        # other set so the chooser lands on the combined one.
        tabs = get_activation_tables(self.m.arch)
        items = []
        for k, v in tabs.items():
            if k != "natural_log_exp_and_others":
                v = {f for f in v if f.name not in ("Exp", "Ln")}
            items.append((k, v))
        _bass_rust.insert_act_table_loads(self, items)

F16 = mybir.dt.float16
F32 = mybir.dt.float32
AF = mybir.ActivationFunctionType
OP = mybir.AluOpType

# --- model config (hardcoded from the problem spec) ---
B, C_IN, H_IN, W_IN = 16, 1, 12, 2500
P_, Q_ = 1, 100
D, NH, DEPTH = 768, 12, 12
GH, GW = 12, 25
L = GH * GW                      # 300
LEN_KEEP = 75
HD = D // NH                     # 64
SCALE = HD ** -0.5               # 0.125
EPS_LN = 1e-5
MLP = 4 * D                      # 3072

NCORES = 8
BL = B // NCORES                 # 2 images per core
KT = 1 + LEN_KEEP                # 76 tokens per image
T = BL * KT                      # 152 token columns per core
NCH = D // 128                   # 6 feature chunks
MCH = MLP // 128                 # 24 mlp chunks
PIX = P_ * Q_                    # 100 pixels per patch


def bfree(ap, n, at=1):
    """Insert a 0-step (broadcast) free dim of size n at position `at`."""
    new_ap = list(ap.ap[:at]) + [[0, n]] + list(ap.ap[at:])
    return bass.AP(tensor=ap.tensor, offset=ap.offset, ap=new_ap)


def build(depth=DEPTH):
    nc = _Bacc("TRN2", target_bir_lowering=False, debug=False,
               num_devices=NCORES)

    # DRAM I/O
    # patches and the patch-embed weight share one input DMA
    pwT = nc.dram_tensor("pwT", [PIX, T + D], F16, kind="ExternalInput").ap()
    posT = nc.dram_tensor("posT", [NCH, 128, T], F32, kind="ExternalInput").ap()
    mvec = nc.dram_tensor("mvec", [BL, KT], F16, kind="ExternalInput").ap()
    wqkvT = nc.dram_tensor("wqkvT", [depth, D, 3 * D], F16,
                           kind="ExternalInput").ap()
    wprojT = nc.dram_tensor("wprojT", [depth, D, D], F16,
                            kind="ExternalInput").ap()
    wfc1T = nc.dram_tensor("wfc1T", [depth, D, MLP], F16, kind="ExternalInput").ap()
    wfc2T = nc.dram_tensor("wfc2T", [depth, MLP, D], F16, kind="ExternalInput").ap()
    # [0:2D]: -colsum(Wq|Wk); [2D:3D]: -colsum(Wv); [3D:]: -colsum(Wfc1),
    # packed two partition rows of 2688 so the tile costs half the columns
    wvecs = nc.dram_tensor("wvecs", [depth, 2, 2688], F16,
                           kind="ExternalInput").ap()
    out_d = nc.dram_tensor("out", [128, NCH, T], F16, kind="ExternalOutput").ap()

    with tile.TileContext(nc) as tc, ExitStack() as ctx:
        pool = lambda name, bufs, **kw: ctx.enter_context(
            tc.tile_pool(name=name, bufs=bufs, **kw))

        const = pool("const", 1)
        hp = pool("hp", 2)
        lnp = pool("lnp", 2)
        qkp = pool("qkp", 1)
        vp = pool("vp", 2)
        ep = pool("ep", 2)
        otp = pool("otp", 1)
        gp = pool("gp", 1)
        tinyp = pool("tinyp", 5)
        medp = pool("medp", 3)
        bcp = pool("bcp", 2)
        wsump = pool("wsump", 2)
        wqkvp = pool("wqkvp", 8)
        wprojp = pool("wprojp", 7)
        wfc1p = pool("wfc1p", 14)
        wfc2p = pool("wfc2p", 6)

        psB = pool("psB", 3, space="PSUM")
        psC = pool("psC", 4, space="PSUM")
        pab = pool("pab", 1, space="PSUM")

        # constants
        ones16 = const.tile([128, 1], F16)
        nc.vector.memset(ones16[:], 1.0)
        onesr = const.tile([1, 64], F16)
        nc.vector.memset(onesr[:], 1.0)
        onesr128 = const.tile([1, 128], F16)
        nc.vector.memset(onesr128[:], 1.0)
        eps_t = const.tile([1, 1], F32)
        nc.vector.memset(eps_t[:], EPS_LN)
        # pre-load the ln/exp table set while the input DMAs stream; the
        # patch-embed squares then run from this set with no reload
        dmy0 = tinyp.tile([1, 1], F16, tag="tiny16")
        nc.scalar.activation(dmy0[:], eps_t[:], AF.Ln)

        # static inputs; the patch-embed-only tiles borrow weight-pool
        # slots (they die right after the patch embed, so the slots return
        # to the steady-state weight stream)
        pw_sb = wsump.tile([PIX, T + D], F16, tag="wvec", name="pw_sb")
        nc.sync.dma_start(out=pw_sb[:], in_=pwT[:])
        pos_sb = wfc2p.tile([128, NCH, T], F32, tag="wfc2", name="pos_sb")
        nc.sync.dma_start(out=pos_sb[:], in_=posT.rearrange("c p t -> p c t"))
        m_sb = const.tile([KT, BL], F16)
        nc.sync.dma_start(out=m_sb[:], in_=mvec.rearrange("b t -> t b"))
        m32_sb = const.tile([KT, BL], F32)
        nc.vector.tensor_copy(m32_sb[:], m_sb[:])

        def drain(ps_aps, h_old, h_new, lnin_new, add_sb=None):
            """Residual drain, both 3-chunk groups: DVE writes the fp16
            matmul copies into lnin_new[:,0] first (the critical
            successors), then squares the second group at fp16 2x rate
            while ACT squares the first; the fp32 stream updates into
            h_new go last (they are not needed until the next drain).
            Pool has no PSUM port on TRN2, so everything lives on DVE/ACT.
            add_sb: use these sbuf tensors instead of h_old (patch embed)."""
            sls = [slice(0, 3), slice(3, 6)]
            srcs = [add_sb[g] if add_sb is not None else h_old[:, sls[g], :]
                    for g in range(2)]
            for g in range(2):
                nc.vector.tensor_add(lnin_new[:, 0, sls[g], :], srcs[g],
                                     ps_aps[g])
                if g == 0:
                    nc.scalar.activation(lnin_new[:, 1, sls[0], :],
                                         lnin_new[:, 0, sls[0], :], AF.Square)
            nc.vector.tensor_mul(lnin_new[:, 1, sls[1], :],
                                 lnin_new[:, 0, sls[1], :],
                                 lnin_new[:, 0, sls[1], :])
            for g in range(2):
                nc.vector.tensor_add(h_new[:, sls[g], :], srcs[g], ps_aps[g])

        def ln_stats(lnin):
            """lnin [128,2,NCH,T] fp16 prefilled with [x | x^2].
            Returns (mu16 [1,T] f16, ab psum [128,2,T] f32 with
            [:,0]=rstd, [:,1]=mu broadcast, anb16 [1,2,T] f16)."""
            st = psC.tile([1, 2, T], F32, tag="psC", name="st")
            for c in range(NCH):
                nc.tensor.matmul(st[:], ones16[:, 0:1], lnin[:, :, c, :],
                                 start=(c == 0), stop=(c == NCH - 1))
            mean = tinyp.tile([1, T], F32, tag="tiny")
            nc.vector.tensor_scalar_mul(mean[:], st[0:1, 0, :], 1.0 / D)
            msq = tinyp.tile([1, T], F32, tag="tiny")
            nc.vector.tensor_mul(msq[:], mean[:], mean[:])
            v = tinyp.tile([1, T], F32, tag="tiny")
            nc.vector.scalar_tensor_tensor(v[:], st[0:1, 1, :], 1.0 / D, msq[:],
                                           op0=OP.mult, op1=OP.subtract)
            # rstd = exp(-0.5*ln(v+eps)) on ACT: Ln and Exp live in the
            # same table set (see _Bacc), and this replaces the 8-op DVE
            # Newton chain (each [1,T] op pays ~150ns of sem latency)
            anb16 = medp.tile([1, 2, T], F16, tag="anb")
            lnv = tinyp.tile([1, T], F32, tag="tiny")
            nc.scalar.activation(lnv[:], v[:], AF.Ln, bias=eps_t[0:1, 0:1])
            nc.scalar.activation(anb16[0:1, 0, :], lnv[:], AF.Exp, scale=-0.5)
            nc.vector.tensor_copy(anb16[0:1, 1, :], mean[:])
            # mu16 on two partition rows (the packed wvec correction rows
            # need a matching rhs base partition)
            # fast path row-0 copy feeds the early corrections; the
            # row-32 replica (packed-wvec fc1 slices) comes from a Pool
            # broadcast that never gates the LN chain
            mu16 = tinyp.tile([1, T], F16, tag="tiny16")
            nc.vector.tensor_copy(mu16[:], mean[:])
            mu32 = tinyp.tile([33, T], F16, tag="mu2")
            nc.gpsimd.partition_broadcast(mu32[:], mu16[:])
            return (mu16, mu32), anb16

        def ln_bcast(anb16):
            """Broadcast [1,2,T] across partitions via PE ones-matmul,
            then ACT-copy PSUM->SBUF (the drains' other operand is PSUM,
            and only one PSUM operand per DVE op is legal)."""
            ab = pab.tile([128, 2, T], F32, tag="pab")
            nc.tensor.matmul(ab[:], onesr128[:], anb16[:],
                             start=True, stop=True)
            ab_sb = bcp.tile([128, 2, T], F16, tag="bc")
            nc.scalar.activation(ab_sb[:], ab[:], AF.Copy)
            return ab_sb

        # residual stream, feature-major fp32 (double-buffered so the
        # Pool-side update overlaps the DVE-side fp16 cast)
        H = hp.tile([128, NCH, T], F32, tag="H")
        lnin1 = lnp.tile([128, 2, NCH, T], F16, tag="lnin")

        # ---- patch embed + pos add ----
        pe_ps = []
        for grp in range(2):
            ps3 = psB.tile([128, 3, T], F32, tag="psB", name="pe3")
            for i in range(3):
                c = 3 * grp + i
                nc.tensor.matmul(ps3[:, i, :],
                                 pw_sb[:, T + 128 * c:T + 128 * (c + 1)],
                                 pw_sb[:, 0:T], start=(i == 0), stop=(i == 2))
            pe_ps.append(ps3)
        drain([p[:, :, :] for p in pe_ps], None, H, lnin1,
              add_sb=[pos_sb[:, 0:3, :], pos_sb[:, 3:6, :]])

        for l in range(depth):
            # weight loads for this layer (emitted first so DMA starts early)
            wvec_t = wsump.tile([33, 2688], F16, tag="wvec")
            nc.sync.dma_start(out=wvec_t[0:33:32, :], in_=wvecs[l])
            # DMA queue is FIFO; emit in expected buffer-free order so a
            # gate-blocked load never shadows an already-loadable one:
            # spare-slot loads first, then by when the prior layer's tiles
            # die (qkv ~35%, proj ~52%, fc1a ~62%, fc1b ~75%, fc2 late).
            wqkv = [wqkvp.tile([128, 3 * D], F16, tag="wqkv", name="wqkv")
                    for _ in range(NCH)]
            wfc1 = [[wfc1p.tile([128, MLP // 2], F16, tag="wfc1", name="wfc1")
                     for _ in range(NCH)] for _ in range(2)]
            wproj = [wprojp.tile([128, D], F16, tag="wproj", name="wproj")
                     for _ in range(NCH)]
            wfc2_t = [wfc2p.tile([128, 6, D], F16, tag="wfc2", name="wfc2")
                      for _ in range(4)]

            def dma_qkv(k):
                nc.sync.dma_start(out=wqkv[k][:],
                                  in_=wqkvT[l, 128 * k:128 * (k + 1), :])

            def dma_fc1(h, k):
                nc.sync.dma_start(out=wfc1[h][k][:],
                                  in_=wfc1T[l, 128 * k:128 * (k + 1),
                                            1536 * h:1536 * (h + 1)])

            def dma_proj(k):
                nc.sync.dma_start(out=wproj[k][:],
                                  in_=wprojT[l, 128 * k:128 * (k + 1), :])

            def dma_fc2(q):
                nc.sync.dma_start(
                    out=wfc2_t[q][:],
                    in_=wfc2T[l, 768 * q:768 * (q + 1), :].rearrange(
                        "(k p) j -> p k j", p=128))

            for k in range(NCH):
                dma_qkv(k)
            dma_fc1(0, 0)
            dma_fc1(0, 1)
            dma_fc2(0)
            dma_fc2(1)
            for k in range(NCH):
                dma_proj(k)
            for k in range(2, NCH):
                dma_fc1(0, k)
            for k in range(NCH):
                dma_fc1(1, k)
            dma_fc2(2)
            dma_fc2(3)

            # ---- LN1 stats (x and x^2 already staged in lnin1) ----
            (mu16_1, mu32_1), anb16_1 = ln_stats(lnin1)

            # ---- QKV: Q,K feature-major on raw x16 + K=1 correction ----
            qk16 = qkp.tile([128, 2 * NCH, T], F16, tag="qk")
            ab1 = None
            first = True
            for grp in [0, 2, 1, 3]:
                ps3 = psB.tile([128, 3, T], F32, tag="psB", name="qk3")
                for i in range(3):
                    oc = 3 * grp + i
                    for k in range(NCH):
                        nc.tensor.matmul(ps3[:, i, :],
                                         wqkv[k][:, 128 * oc:128 * (oc + 1)],
                                         lnin1[:, 0, k, :],
                                         start=(k == 0), stop=False)
                    nc.tensor.matmul(ps3[:, i, :],
                                     wvec_t[0:1, 128 * oc:128 * (oc + 1)],
                                     mu16_1[:], start=False, stop=True)
                if first:
                    # emitted after the first group's matmuls: the PE sits
                    # behind them in queue order, so the wait on the DVE
                    # stats chain overlaps that GEMM stream
                    ab1 = ln_bcast(anb16_1)
                    first = False
                nc.vector.tensor_mul(qk16[:, 3 * grp:3 * (grp + 1), :],
                                     ps3[:, :, :], bfree(ab1[:, 0, :], 3))

            # per-image rstd as a column (PE transpose) * mask -> V scale
            csc = []
            for b in range(BL):
                vt = psC.tile([KT, 1], F16, tag="psC", name="vt")
                nc.tensor.matmul(vt[:], anb16_1[0:1, 0, KT * b:KT * (b + 1)],
                                 ones16[0:1, 0:1], is_transpose=True)
                cs = tinyp.tile([KT, 1], F32, tag="csc")
                nc.vector.tensor_mul(cs[:], vt[:], m32_sb[:, b:b + 1])
                csc.append(cs)

            # ---- V token-major per image, raw x16 + correction ----
            v16 = []
            for b in range(BL):
                vps0 = psC.tile([KT, 384], F32, tag="psC", name="vps")
                vps1 = psC.tile([KT, 384], F32, tag="psC", name="vps")
                for k in range(NCH):
                    nc.tensor.matmul(vps0[:],
                                     lnin1[:, 0, k, KT * b:KT * (b + 1)],
                                     wqkv[k][:, 2 * D:2 * D + 384],
                                     start=(k == 0), stop=False)
                nc.tensor.matmul(vps0[:], mu16_1[0:1, KT * b:KT * (b + 1)],
                                 wvec_t[0:1, 2 * D:2 * D + 384],
                                 start=False, stop=True)
                for k in range(NCH):
                    nc.tensor.matmul(vps1[:],
                                     lnin1[:, 0, k, KT * b:KT * (b + 1)],
                                     wqkv[k][:, 2 * D + 384:3 * D],
                                     start=(k == 0), stop=False)
                nc.tensor.matmul(vps1[:], mu16_1[0:1, KT * b:KT * (b + 1)],
                                 wvec_t[0:1, 2 * D + 384:3 * D],
                                 start=False, stop=True)
                v = vp.tile([KT, D], F16, tag="v")
                nc.vector.tensor_scalar_mul(v[:, 0:384], vps0[:], csc[b][:])
                nc.vector.tensor_scalar_mul(v[:, 384:768], vps1[:], csc[b][:])
                v16.append(v)

            # ---- attention; images interleaved, heads grouped by parity.
            # PV runs on the raw exp(S) values: the attn mask is folded into
            # the V drain (row scale) and the 1/rowsum normalization into the
            # ot16 assembly multiply, so the softmax scalar chain never
            # blocks the PE stream.
            # fully pipelined per-(image, head-group) units: each unit's
            # exp->rowsum->recip->mask->bcast->assembly chain hides behind
            # the later units' S/PV matmul stream.  No softmax eps: rowsums
            # of exp() are bounded >= e^-O(1) here, and the reference's
            # +1e-9 shifts them by ~1e-11 relative.
            ot16 = otp.tile([128, NCH, T], F16, tag="ot")
            e16s = [ep.tile([KT, 2, 6 * KT], F16, tag="e", name="e16")
                    for _ in range(BL)]
            rrs = [medp.tile([1, 2, 6 * KT], F16, tag="rr", name="rr")
                   for _ in range(BL)]
            rrbs = [bcp.tile([64, 2, 6 * KT], F16, tag="rb", name="rrb")
                    for _ in range(BL)]
            units = [(b, g) for b in range(BL) for g in range(2)]

            def att_s(b, g):
                sps = psC.tile([KT, 6 * KT], F32, tag="psC", name="sps")
                for j in range(6):
                    nc.tensor.matmul(
                        sps[:, KT * j:KT * (j + 1)],
                        qk16[64 * g:64 * (g + 1), 6 + j, KT * b:KT * (b + 1)],
                        qk16[64 * g:64 * (g + 1), j, KT * b:KT * (b + 1)],
                        start=True, stop=True)
                nc.scalar.activation(e16s[b][:, g, :], sps[:],
                                     AF.Exp, scale=SCALE)

            def att_norm(b, g):
                rps = psC.tile([1, 6 * KT], F32, tag="psC", name="rps")
                nc.tensor.matmul(rps[:], m_sb[:, b:b + 1],
                                 e16s[b][:, g, :], start=True, stop=True)
                with nc.allow_low_precision(reason="softmax norm fp16"):
                    nc.vector.reciprocal(rrs[b][0:1, g, :], rps[:])

            def att_out(b, g):
                ops = psC.tile([64, 6 * KT], F32, tag="psC", name="ops")
                for j in range(6):
                    nc.tensor.matmul(
                        ops[:, KT * j:KT * (j + 1)],
                        v16[b][:, 128 * j + 64 * g:128 * j + 64 * g + 64],
                        e16s[b][:, g, KT * j:KT * (j + 1)],
                        start=True, stop=True)
                rbp = pab.tile([64, 6 * KT], F32, tag="pab", name="rbp")
                nc.tensor.matmul(rbp[:], onesr[:], rrs[b][0:1, g, :],
                                 start=True, stop=True)
                nc.scalar.activation(rrbs[b][:, g, :], rbp[:], AF.Copy)
                nc.vector.tensor_mul(
                    ot16[64 * g:64 * (g + 1), :, KT * b:KT * (b + 1)],
                    ops[:].rearrange("p (j t) -> p j t", j=6),
                    rrbs[b][:, g, :].rearrange("p (j t) -> p j t", j=6))

            # 3-stage software pipeline: slot i runs S/exp of unit i, the
            # rowsum/recip of unit i-1, and the PV/normalize of unit i-2,
            # so every unit's scalar chain hides behind later units' PE work
            for i in range(len(units) + 2):
                if i < len(units):
                    att_s(*units[i])
                if i == 1:
                    # hoist the gelu-set load into the attention phase
                    dmy = tinyp.tile([1, 1], F16, tag="tiny16")
                    nc.scalar.activation(dmy[:], ones16[0:1, 0:1], AF.Gelu)
                if 1 <= i < len(units) + 1:
                    att_norm(*units[i - 1])
                if 2 <= i:
                    att_out(*units[i - 2])

            # ---- proj + residual ----
            # proj split by image: img0's half streams on PE while img1's
            # softmax scalar chain is still finishing
            pj = [psB.tile([128, 3, T], F32, tag="psB", name="pj3")
                  for _ in range(2)]
            for b in range(BL):
                cs = slice(KT * b, KT * (b + 1))
                for grp in range(2):
                    for i in range(3):
                        oc = 3 * grp + i
                        for k in range(NCH):
                            nc.tensor.matmul(pj[grp][:, i, cs],
                                             wproj[k][:, 128 * oc:
                                                      128 * (oc + 1)],
                                             ot16[:, k, cs],
                                             start=(k == 0 and b == 0 and i == 0),
                                             stop=(k == NCH - 1 and b == BL - 1
                                                   and i == 2))
            Hn = hp.tile([128, NCH, T], F32, tag="H")
            lnin2 = lnp.tile([128, 2, NCH, T], F16, tag="lnin")
            drain([pj[0][:, :, :], pj[1][:, :, :]], H, Hn, lnin2)
            H = Hn

            # ---- LN2 + MLP ----
            (mu16_2, mu32_2), anb16_2 = ln_stats(lnin2)
            g16 = gp.tile([128, MCH, T], F16, tag="g")
            ab2 = None
            for grp in range(MCH // 3):
                ps3 = psB.tile([128, 3, T], F32, tag="psB")
                for i in range(3):
                    oc = 3 * grp + i
                    for k in range(NCH):
                        h, col = divmod(128 * oc, 1536)
                        nc.tensor.matmul(ps3[:, i, :],
                                         wfc1[h][k][:, col:col + 128],
                                         lnin2[:, 0, k, :],
                                         start=(k == 0), stop=False)
                    row, coff = divmod(3 * D + 128 * oc, 2688)
                    mu_ap = mu16_2[0:1, :] if row == 0 else mu32_2[32:33, :]
                    nc.tensor.matmul(ps3[:, i, :],
                                     wvec_t[32 * row:32 * row + 1,
                                            coff:coff + 128],
                                     mu_ap, start=False, stop=True)
                if grp == 0:
                    ab2 = ln_bcast(anb16_2)
                nc.vector.tensor_mul(ps3[:, :, :], ps3[:, :, :],
                                     bfree(ab2[:, 0, :], 3))
                nc.scalar.activation(g16[:, 3 * grp:3 * (grp + 1), :], ps3[:, :, :],
                                     AF.Gelu)
            # hoist the exp-set load into the MLP phase (covers the next
            # layer's attention exps and this layer's trailing squares --
            # square lives in every set)
            dmy = tinyp.tile([1, 1], F16, tag="tiny16")
            nc.scalar.activation(dmy[:], ones16[0:1, 0:1], AF.Exp)
            # fc2 with k OUTER in halves so weight k-tiles die right after
            # use and next-layer DMA streams during this stage; two psB
            # accumulator tiles (oc 0-2 / 3-5) drain progressively.
            acc = [psB.tile([128, 3, T], F32, tag="psB", name="acc2")
                   for _ in range(2)]
            Hn = hp.tile([128, NCH, T], F32, tag="H")
            lnin_n = lnp.tile([128, 2, NCH, T], F16, tag="lnin")
            KH = MCH // 2
            for half in range(2):
                for oc in range(NCH):
                    for kk in range(KH):
                        k = half * KH + kk
                        nc.tensor.matmul(acc[oc // 3][:, oc % 3, :],
                                         wfc2_t[k // 6][:, k % 6,
                                                        128 * oc:128 * (oc + 1)],
                                         g16[:, k, :],
                                         start=(k == 0 and oc % 3 == 0),
                                         stop=(k == MCH - 1 and oc % 3 == 2))
                    if half == 1 and oc == 2:
                        # bank A closed: start its fp16 cast + ACT square
                        # while bank B still accumulates on PE
                        nc.vector.tensor_add(lnin_n[:, 0, 0:3, :],
                                             H[:, 0:3, :], acc[0][:, :, :])
                        nc.scalar.activation(lnin_n[:, 1, 0:3, :],
                                             lnin_n[:, 0, 0:3, :], AF.Square)
            nc.vector.tensor_add(lnin_n[:, 0, 3:6, :], H[:, 3:6, :],
                                 acc[1][:, :, :])
            nc.vector.tensor_mul(lnin_n[:, 1, 3:6, :], lnin_n[:, 0, 3:6, :],
                                 lnin_n[:, 0, 3:6, :])
            for g in range(2):
                nc.vector.tensor_add(Hn[:, 3 * g:3 * g + 3, :],
                                     H[:, 3 * g:3 * g + 3, :],
                                     acc[g][:, :, :])
            H = Hn
            lnin1 = lnin_n

        # ---- final LN (fp16 out; host upcasts) + store ----
        (mu16_f, mu32_f), anb16_f = ln_stats(lnin1)
        abf = ln_bcast(anb16_f)
        yf = otp.tile([128, NCH, T], F16, tag="ot", name="yf")
        for grp in range(2):
            sl = slice(3 * grp, 3 * (grp + 1))
            nc.vector.scalar_tensor_tensor(yf[:, sl, :], lnin1[:, 0, sl, :], 1.0,
                                           bfree(abf[:, 1, :], 3),
                                           op0=OP.mult, op1=OP.subtract)
            nc.vector.tensor_mul(yf[:, sl, :], yf[:, sl, :],
                                 bfree(abf[:, 0, :], 3))
            nc.sync.dma_start(out=out_d[:, sl, :], in_=yf[:, sl, :])

    nc.compile()
    return nc


def prep_inputs(inputs, depth=DEPTH):
    """Host-side marshalling. Returns per-core in_maps list."""
    g = {k: np.asarray(v) for k, v in inputs.items()}
    x = g["x"].astype(np.float32)
    noise = g["noise"].astype(np.float32)
    attn_mask = g["attn_mask"].astype(np.float32)
    ids_y = g["pos_embed_y_ids"].astype(np.int64)

    ids_shuffle = np.argsort(noise, axis=1, kind="stable")
    ids_keep = ids_shuffle[:, :LEN_KEEP]                      # (B, 75)

    patches = x.reshape(B, GH, GW, Q_).reshape(B, L, Q_)      # (B, 300, 100)
    mask_l = attn_mask.reshape(B, L)

    # pos vector per patch: [pos_y(384) | pos_x(384) * mask]
    pos_y = g["pos_y_table"].astype(np.float32)               # (13, 384)
    pos_x = g["pos_embed_x"].astype(np.float32)[0]            # (26, 384)
    ids_y_l = ids_y.reshape(B, L)
    gw_idx = np.tile(np.arange(GW), GH)                       # (300,)
    pos_full = np.zeros((B, L, D), np.float32)
    pos_full[:, :, :D // 2] = pos_y[ids_y_l]
    pos_full[:, :, D // 2:] = mask_l[:, :, None] * pos_x[gw_idx + 1][None]

    cls_vec = g["cls_token"].astype(np.float32).reshape(D).copy()
    cls_vec[D // 2:] += pos_x[0]

    wqkvT = np.ascontiguousarray(
        g["qkv_w"].astype(np.float32).transpose(0, 2, 1)[:depth]).astype(np.float16)
    wprojT = np.ascontiguousarray(
        g["proj_w"].astype(np.float32).transpose(0, 2, 1)[:depth]).astype(np.float16)
    wfc1T = np.ascontiguousarray(
        g["fc1_w"].astype(np.float32).transpose(0, 2, 1)[:depth]).astype(np.float16)
    wfc2T = np.ascontiguousarray(
        g["fc2_w"].astype(np.float32).transpose(0, 2, 1)[:depth]).astype(np.float16)
    wpatchT = np.ascontiguousarray(
        g["conv_w"].astype(np.float32).reshape(D, Q_).T).astype(np.float16)

    wsqn = -wqkvT.astype(np.float32).sum(axis=1).astype(np.float16)  # (depth, 3D)
    wsf1n = -wfc1T.astype(np.float32).sum(axis=1).astype(np.float16)
    wvecs = np.ascontiguousarray(np.concatenate([wsqn, wsf1n], axis=1)
                                 .reshape(depth, 2, 2688))

    in_maps = []
    for core in range(NCORES):
        patchesT = np.zeros((PIX, T), np.float16)
        posT = np.zeros((D, T), np.float32)
        mv = np.zeros((BL, KT), np.float16)
        for b in range(BL):
            img = core * BL + b
            sel = ids_keep[img]                               # (75,)
            patchesT[:, KT * b + 1:KT * (b + 1)] = patches[img, sel].T
            posT[:, KT * b] = cls_vec
            posT[:, KT * b + 1:KT * (b + 1)] = pos_full[img, sel].T
            mv[b, 0] = 1.0
            mv[b, 1:] = mask_l[img, np.sort(sel)]
        in_maps.append({
            "pwT": np.concatenate([patchesT, wpatchT], axis=1),
            "posT": posT.reshape(NCH, 128, T),
            "mvec": mv,
            "wqkvT": wqkvT,
            "wprojT": wprojT,
            "wfc1T": wfc1T,
            "wfc2T": wfc2T,
            "wvecs": wvecs,
        })
    return in_maps


_NC_CACHE = {}


def kernel(**inputs):
    if "nc" not in _NC_CACHE:
        _NC_CACHE["nc"] = build()
    nc = _NC_CACHE["nc"]
    in_maps = prep_inputs(inputs)
    res = run_bass_kernel_spmd(nc, in_maps, list(range(NCORES)))
    # device output is feature-major [p, c, t] with feature = 128*c + p
    outs = []
    for i in range(NCORES):
        a = res.results[i]["out"].reshape(128, NCH, T).astype(np.float32)
        a = a.transpose(1, 0, 2).reshape(D, T)
        outs.append(np.ascontiguousarray(a.T).reshape(BL, KT, D))
    return np.concatenate(outs, axis=0).astype(np.float32)
